# revision 83
# baseline (speedup 1.0000x reference)
"""Trainium2 Bass kernel: multi-head attention forward (B=2, S=2048, D=1024, H=16).

Sharding: 8 cores = data-parallel over batch (2) x tensor-parallel over heads
(4 head-groups of 4 heads).  Host sums the 4 partial outputs per batch and
adds the bias.

Per-core algorithm (all matmul operands bf16: 1 cycle/row at any width):
  qT/kT = w.T @ x.T        [256, S]   (per-head-group projections)
  v     = x @ wv           [S, 256]   (+ a ones column per head for the
                                       softmax denominators)
  per (query-block j, head h, key-tile ski):
      scoresT[sk, sq] = k_h @ q_h.T   (psum, 512-wide chunks, causal-trimmed)
      attnT = exp(scale * scoresT)    (ACT, psum -> SBUF bf16)
      causal mask on the diagonal tile (GPSIMD affine_select, fill 0)
  per query-tile qc (once its diagonal exp is done):
      ctx'[sq, 65] = sum_ski attnT_tile.T @ [v_h | 1]   (keys contracted on
          partitions, queries on output partitions: 65 cycles per key-tile
          instead of the 512 a [65, sq]-oriented PV pays)
      recip = 1/ctx'[:, 64]           (DVE approx; SBUF source only)
      ctx_n[sq, 64] = ctx' * recip    (per-partition scalar, bf16)
  per head-pair: PE-transpose ctx_n [sq,128] -> ctxT [128, sq] for the
      output projection (row-sharded wo), emitted per 128-row s-tile so the
      projection and its DMA overlap the remaining attention.
"""

import sys

sys.path.insert(0, "/opt/trn_rl_repo")

import numpy as np

B, S, D = 2, 2048, 1024
H = 16
DH = 64
HL = 4  # heads per core
NCORES = 8
W_SCALE = 32.0  # fp8 weight prescale so w ~ N(0, 0.02) clears e4m3 denormals

_PROGRAM_CACHE = {}


def build_program(S=S, D=D, HL=HL, DH=DH):
    from collections import deque

    import concourse.tile as tile
    from concourse import bacc, mybir

    f32 = mybir.dt.float32
    bf16 = mybir.dt.bfloat16
    f8 = mybir.dt.float8e4
    A = mybir.ActivationFunctionType
    Alu = mybir.AluOpType
    DR = mybir.MatmulPerfMode.DoubleRow

    KD = D // 128        # contraction chunks for the projections
    KD4 = D // 256       # fp8 DoubleRow contraction chunks (2 rows/partition)
    M = HL * DH          # per-core projected width (256)
    MQ = M // 128        # qT/kT partition tiles (2)
    ST = S // 128        # 128-row s tiles
    W = min(1024, S)     # query-block width
    NJ = S // W          # query blocks
    TPB = W // 128       # 128-tiles per query block
    NCH = S // 512       # 512-wide s chunks
    GA_N = min(2, NCH)   # s chunks covered by the k-outer warmup group
    scale = 1.0 / float(np.sqrt(DH))

    nc = bacc.Bacc("TRN2", target_bir_lowering=False, debug=False)
    xT = nc.dram_tensor("xT", (D, S), bf16, kind="ExternalInput").ap()
    # q/k path in fp8e4m3: weights host-prescaled by W_SCALE, undone in the
    # psum->sbuf copies; v/out stay bf16 (fp8 there fails the 2e-2 gate).
    # wq|wk are host-interleaved into one tensor so the DMA rows reach the
    # 512B no-penalty descriptor size.
    x8T = nc.dram_tensor("x8T", (D, S), f8, kind="ExternalInput").ap()
    wqk = nc.dram_tensor("wqk", (D, 2 * M), f8, kind="ExternalInput").ap()
    wv = nc.dram_tensor("wv", (D, M), bf16, kind="ExternalInput").ap()
    wo = nc.dram_tensor("wo", (M, D), bf16, kind="ExternalInput").ap()
    out = nc.dram_tensor("out", (S, D), bf16, kind="ExternalOutput").ap()

    with tile.TileContext(nc) as tc:
        with (
            tc.tile_pool(name="persist", bufs=1) as mpool,
            tc.tile_pool(name="ostage", bufs=3) as ostage,
            tc.tile_pool(name="rp", bufs=2) as rpool,
        ):
            wo_sb = mpool.tile([128, MQ, D], bf16, tag="wo")
            qT_sb = mpool.tile([128, MQ, S], bf16, tag="qT")
            kT_sb = mpool.tile([128, MQ, S], bf16, tag="kT")
            v_sb = mpool.tile([128, ST, HL, DH + 1], bf16, tag="v")
            ctx_sb = mpool.tile([128, MQ, S], bf16, tag="ctx")
            attn_sb = mpool.tile([128, 3, ST, W], bf16, tag="attn")
            ctxn_sb = mpool.tile([128, MQ, TPB, 128], bf16, tag="ctxn")
            ident = mpool.tile([128, 128], bf16, tag="id")
            xt = mpool.tile([128, KD, S], bf16, tag="xt")
            x8 = mpool.tile([128, KD4, 2, S], f8, tag="x8")
            wqk_sb = mpool.tile([128, KD4, 2, 2 * M], f8, tag="wqk")
            wv_sb = mpool.tile([128, KD, M], bf16, tag="wv")

            nc.gpsimd.memset(ident[:], 0.0)
            nc.gpsimd.affine_select(
                out=ident[:], in_=ident[:], compare_op=Alu.not_equal,
                fill=1.0, base=0, pattern=[[-1, 128]], channel_multiplier=1,
            )
            # ones columns for the PV denominator trick
            nc.gpsimd.memset(v_sb[:, :, :, DH], 1.0)

            # ---------------- loads (order gates the exp-stream start) ----
            wqk_r = wqk.rearrange("(c i p) m -> p c i m", p=128, i=2)
            wv_r = wv.rearrange("(k p) m -> p k m", p=128)
            xT_r = xT.rearrange("(k p) s -> p k s", p=128)
            x8_r = x8T.rearrange("(c i p) s -> p c i s", p=128, i=2)
            nhalf = 2 if NCH >= 4 else 1
            xh = S // nhalf
            nq = 4 if NCH >= 4 else 1
            xq = S // nq
            nc.sync.dma_start(wqk_sb[:], wqk_r[:])
            nc.sync.dma_start(x8[:, 0:KD4 // 2, :, 0:xh],
                              x8_r[:, 0:KD4 // 2, :, 0:xh])
            nc.sync.dma_start(x8[:, KD4 // 2:KD4, :, 0:xh],
                              x8_r[:, KD4 // 2:KD4, :, 0:xh])
            nc.sync.dma_start(xt[:, :, 0:xq], xT_r[:, :, 0:xq])
            nc.sync.dma_start(wv_sb[:], wv_r[:])
            if nq > 1:
                nc.sync.dma_start(xt[:, :, xq:2 * xq], xT_r[:, :, xq:2 * xq])
            if nhalf > 1:
                nc.sync.dma_start(x8[:, :, :, xh:S], x8_r[:, :, :, xh:S])
            if nq > 1:
                nc.sync.dma_start(xt[:, :, 2 * xq:3 * xq],
                                  xT_r[:, :, 2 * xq:3 * xq])
                nc.sync.dma_start(xt[:, :, 3 * xq:S], xT_r[:, :, 3 * xq:S])
            nc.sync.dma_start(wo_sb[:], wo.rearrange("(k p) d -> p k d", p=128))

            # ---------- warmup projections (c-outer over GA_N chunks) -----
            # q/k for all heads, s < GA_N*512: enough to start attention
            # (j=0, both passes) as soon as the fp8 first half of x lands.
            n_ga = (GA_N + 1) * MQ
            ga_cm = tc.tile_pool(name="gaps", bufs=n_ga, space="PSUM")
            gapool = ga_cm.__enter__()
            ga = [gapool.tile([128, 512], f32, tag="ga", name=f"ga{i}")
                  for i in range(n_ga)]
            # first-half-of-x chunks sweep across all tiles (DMA-paced),
            # then each tile finishes its second half and copies out
            # immediately, so the copies gating the first exps retire early
            descale = 1.0 / W_SCALE
            ga_tiles = []
            i = 0
            for m in range(MQ):
                for n in range(GA_N):
                    sl = slice(n * 512, (n + 1) * 512)
                    ga_tiles.append((ga[i], m * 128, qT_sb, m, sl, True))
                    i += 1
                    if n == 0:  # later kT chunks are deferred units
                        ga_tiles.append((ga[i], M + m * 128, kT_sb, m, sl,
                                         False))
                        i += 1
            for c in range(KD4 // 2):
                for t, col, dst, m, sl, is_q in ga_tiles:
                    nc.tensor.matmul(t[:], wqk_sb[:, c, :, col:col + 128],
                                     x8[:, c, :, sl], perf_mode=DR,
                                     start=(c == 0), stop=False)
            for t, col, dst, m, sl, is_q in ga_tiles:
                for c in range(KD4 // 2, KD4):
                    nc.tensor.matmul(t[:], wqk_sb[:, c, :, col:col + 128],
                                     x8[:, c, :, sl], perf_mode=DR,
                                     start=False, stop=(c == KD4 - 1))
                if is_q:
                    nc.vector.tensor_scalar(dst[:, m, sl], t[:],
                                            descale, None, Alu.mult)
                else:
                    nc.scalar.activation(dst[:, m, sl], t[:], A.Copy,
                                         scale=descale)
            ga_cm.__exit__(None, None, None)

            # --------------- attention-phase psum pools -------------------
            # entry order fixes bank placement: the scratch ring lands on the
            # banks whose warmup copies retire first, the scores ring next
            scr_cm = tc.tile_pool(name="scrps", bufs=2, space="PSUM")
            scrpool = scr_cm.__enter__()
            sc_cm = tc.tile_pool(name="scps", bufs=2, space="PSUM")
            spool = sc_cm.__enter__()
            ctx_cm = tc.tile_pool(name="ctxps", bufs=2, space="PSUM")
            cpool = ctx_cm.__enter__()

            # ------------- deferred projection units (one psum tile) ------
            def qk_unit(base, dst, m, n):
                def emit():
                    ps = scrpool.tile([128, 512], f32, tag="scr")
                    sl = slice(n * 512, (n + 1) * 512)
                    col = base + m * 128
                    for c in range(KD4):
                        nc.tensor.matmul(ps[:], wqk_sb[:, c, :, col:col + 128],
                                         x8[:, c, :, sl], perf_mode=DR,
                                         start=(c == 0), stop=(c == KD4 - 1))
                    nc.vector.tensor_scalar(dst[:, m, sl], ps[:],
                                            1.0 / W_SCALE, None, Alu.mult)
                return emit

            def v_unit(st):
                def emit():
                    ps = scrpool.tile([128, 512], f32, tag="scr")
                    for k in range(KD):
                        nc.tensor.matmul(ps[:, 0:M], xt[:, k, st * 128:(st + 1) * 128],
                                         wv_sb[:, k, :], start=(k == 0),
                                         stop=(k == KD - 1))
                    nc.vector.tensor_copy(
                        v_sb[:, st, :, 0:DH],
                        ps[:, 0:M].rearrange("p (h c) -> p h c", h=HL),
                    )
                return emit

            # early units feed j=0 (and j=1's first scores); late units are
            # only needed from j=1's deeper ski range on and fill j=1's
            # PE slack while ACT paces the exp stream
            # j=0 unit schedule keyed by (pass, ski), honoring deadlines:
            # kT n>=1 chunks before scores ski=4n, v(qc) before its burst at
            # step qc+1, qT chunks for j=1 anytime within pass B
            j0sched = {}

            def sched(p, ski, fn):
                j0sched.setdefault((p, ski), []).append(fn)

            for n in range(1, GA_N):
                for m in range(MQ):
                    sched(0, 0, qk_unit(M, kT_sb, m, n))
            # post-slot pops: v(qc) lands after step qc's emission, ahead of
            # its burst in step qc+1
            for qc in range(min(ST, TPB)):
                sched(0, qc, v_unit(qc))
            qn = [qk_unit(0, qT_sb, m, n)
                  for n in range(GA_N, NCH) for m in range(MQ)]
            for i, fn in enumerate(qn):
                sched(1, (2 * i) % TPB, fn)
            late = deque()
            if NJ > 1:
                # interleave the late v tiles with the late-needed kT
                # chunks: v(TPB+i) is read from burst step TPB+i on, kT
                # chunk n from scores step 4n on
                lv = [v_unit(st) for st in range(TPB, ST)]
                lk = [qk_unit(M, kT_sb, m, n)
                      for n in range(GA_N, NCH) for m in range(MQ)]
                while lv or lk:
                    if lv:
                        late.append(lv.pop(0))
                    if lk:
                        late.append(lk.pop(0))
            else:
                for st in range(TPB, ST):
                    sched(1, (2 * st) % TPB, v_unit(st))

            # ------------- output projection for one s-tile ---------------
            def emit_outproj_st(st, act_share):
                o_sb = ostage.tile([128, D], bf16, tag="o")
                for n in range(D // 512):
                    ops = scrpool.tile([128, 512], f32, tag="scr")
                    for p2 in range(MQ):
                        nc.tensor.matmul(
                            ops[:], ctx_sb[:, p2, st * 128:(st + 1) * 128],
                            wo_sb[:, p2, n * 512:(n + 1) * 512],
                            start=(p2 == 0), stop=(p2 == MQ - 1),
                        )
                    sl = slice(n * 512, (n + 1) * 512)
                    if n < D // 512 - act_share:
                        nc.vector.tensor_copy(o_sb[:, sl], ops[:])
                    else:
                        nc.scalar.activation(o_sb[:, sl], ops[:], A.Copy)
                nc.sync.dma_start(out[st * 128:(st + 1) * 128, :], o_sb[:])

            # ------------- attention for one (j, h) -----------------------
            outproj_q = deque()

            def emit_burst(j, h, qc, bank):
                """Flipped-PV burst for query-tile qc + softmax normalize;
                on the second head of a pair also transpose ctx to ctxT and
                queue the s-tile's output projection."""
                hm, po = h // 2, 64 * (h % 2)
                qt = j * TPB + qc
                ctx_ps = cpool.tile([128, DH + 1], f32, tag="ctx")
                for s2 in range(qt + 1):
                    nc.tensor.matmul(
                        ctx_ps[:],
                        attn_sb[:, bank, s2, qc * 128:(qc + 1) * 128],
                        v_sb[:, s2, h, :],
                        start=(s2 == 0), stop=(s2 == qt),
                    )
                dn = rpool.tile([128, 1], f32, tag="dn")
                rc = rpool.tile([128, 1], f32, tag="rc")
                nc.vector.tensor_copy(dn[:], ctx_ps[:, DH:DH + 1])
                # custom-DVE reciprocal must read SBUF (psum source returns
                # garbage on hardware); ~51 ULP is negligible
                nc.vector.reciprocal_approx_fast(out=rc[:], in_=dn[:])
                nc.vector.tensor_scalar(
                    ctxn_sb[:, hm, qc, po:po + DH], ctx_ps[:, 0:DH],
                    rc[:], None, Alu.mult,
                )
                if po:  # both heads of the pair done for this qc
                    tp = scrpool.tile([128, 512], f32, tag="scr")
                    tpb = tp[:, 0:64].bitcast(bf16)
                    nc.tensor.transpose(tpb, ctxn_sb[:, hm, qc, :], ident[:])
                    nc.vector.tensor_copy(
                        ctx_sb[:, hm, qt * 128:(qt + 1) * 128], tpb,
                    )
                    if hm == MQ - 1:
                        outproj_q.append(qt)

            def attention_pass(j, hs, pass_idx):
                """One ski sweep emitting two heads per step (one per
                attn bank/head-pair); bursts lag their diagonal exp by one
                step so the PE never queues a burst ahead of the next
                scores.  attn banks rotate mod 3 so a pass's first exps
                never WAR-wait on the previous pass's last bursts."""
                banks = {hh: (2 * pass_idx + i) % 3 for i, hh in enumerate(hs)}
                nski = TPB * (j + 1)
                for ski in range(nski):
                    # "pre" fillers (projection units) must precede the step
                    # (write-before-read in the PE queue); "post" fillers
                    # (output projections) go behind the step's scores so
                    # they never delay the exp stream
                    yield "pre", ski
                    ex0 = max(0, 128 * ski - j * W)
                    for h in hs:
                        hm, po = h // 2, 64 * (h % 2)
                        qrow = slice(po, po + DH)
                        sc = spool.tile([128, W], f32, tag="sc")
                        for n in range(ex0 // 512, W // 512):
                            lo = max(ex0, n * 512)
                            nc.tensor.matmul(
                                sc[:, lo:(n + 1) * 512],
                                kT_sb[qrow, hm, ski * 128:(ski + 1) * 128],
                                qT_sb[qrow, hm,
                                      j * W + lo: j * W + (n + 1) * 512],
                                start=True, stop=True,
                            )
                        at = attn_sb[:, banks[h], ski]
                        nc.scalar.activation(at[:, ex0:W], sc[:, ex0:W],
                                             A.Exp, scale=scale)
                        if 128 * ski >= j * W:  # diagonal-crossing tile
                            nc.gpsimd.affine_select(
                                out=at[:, ex0:ex0 + 128],
                                in_=at[:, ex0:ex0 + 128],
                                compare_op=Alu.is_ge, fill=0.0,
                                base=j * W + ex0 - 128 * ski,
                                pattern=[[1, 128]], channel_multiplier=-1,
                            )
                    if ski > j * TPB:
                        for h in hs:
                            emit_burst(j, h, ski - 1 - j * TPB, banks[h])
                    yield "post", ski
                for h in hs:
                    emit_burst(j, h, TPB - 1, banks[h])

            for j in range(NJ):
                if j > 0:
                    for fns in [j0sched.pop(k) for k in sorted(j0sched)]:
                        for fn in fns:
                            fn()
                nski_j = TPB * (j + 1)
                for p in range(2):
                    hs = (0, 2) if p == 0 else (1, 3)
                    if j > 0 and p == 1:
                        while late:
                            late.popleft()()
                    popped = False
                    for phase, ski in attention_pass(j, hs, 2 * j + p):
                        if phase == "pre":
                            popped = False
                        elif j == 0 and (p, ski) in j0sched:
                            popped = True
                            for fn in j0sched.pop((p, ski)):
                                fn()
                        elif j > 0 and p == 0 and late:
                            popped = True
                            late.popleft()()
                        elif not popped and outproj_q and (
                            (j == NJ - 1 and p == 1
                             and (ski % 2 == 0 or ski >= nski_j - 4))
                            or (j > 0 and ski % 3 == 2)
                        ):
                            tail = j == NJ - 1 and p == 1 and ski >= nski_j - 4
                            emit_outproj_st(outproj_q.popleft(),
                                            act_share=1 if tail else 0)
            while late:
                late.popleft()()
            while outproj_q:
                emit_outproj_st(outproj_q.popleft(), act_share=1)
            ctx_cm.__exit__(None, None, None)
            sc_cm.__exit__(None, None, None)
            scr_cm.__exit__(None, None, None)

    nc.compile()
    return nc


def _get_program():
    key = (S, D, HL, DH)
    if key not in _PROGRAM_CACHE:
        _PROGRAM_CACHE[key] = build_program(*key)
    return _PROGRAM_CACHE[key]


def prepare_core_inputs(xT, wq, wk, wv, wo):
    """fp32 per-core inputs (xT [D,S], wq/wk/wv [D,M], wo [M,D]) -> the
    device tensor map (bf16 + prescaled-fp8 q/k path)."""
    import ml_dtypes

    bf16 = ml_dtypes.bfloat16
    f8 = ml_dtypes.float8_e4m3
    xT = np.ascontiguousarray(xT, dtype=np.float32)
    wqk = np.concatenate([np.asarray(wq), np.asarray(wk)], axis=1) * W_SCALE
    return {
        "xT": xT.astype(bf16),
        "x8T": xT.astype(f8),
        "wqk": np.ascontiguousarray(wqk).astype(f8),
        "wv": np.ascontiguousarray(wv).astype(bf16),
        "wo": np.ascontiguousarray(wo).astype(bf16),
    }


def make_in_maps(x, Wq, Wk, Wv, Wo):
    x = np.asarray(x, dtype=np.float32)
    Wq = np.asarray(Wq, dtype=np.float32)
    Wk = np.asarray(Wk, dtype=np.float32)
    Wv = np.asarray(Wv, dtype=np.float32)
    Wo = np.asarray(Wo, dtype=np.float32)
    in_maps = []
    for c in range(NCORES):
        b, g = divmod(c, NCORES // B)
        sl = slice(HL * DH * g, HL * DH * (g + 1))
        in_maps.append(
            prepare_core_inputs(x[b].T, Wq[sl, :].T, Wk[sl, :].T,
                                Wv[sl, :].T, Wo[:, sl].T)
        )
    return in_maps


def kernel(x, Wq, Wk, Wv, Wo, bo):
    from concourse import bass2jax

    nc = _get_program()
    in_maps = make_in_maps(x, Wq, Wk, Wv, Wo)
    res = bass2jax.run_bass_via_pjrt(nc, in_maps, n_cores=NCORES)
    outs = [np.asarray(res[c]["out"], dtype=np.float32) for c in range(NCORES)]
    gpb = NCORES // B
    o = np.stack([sum(outs[b * gpb + g] for g in range(gpb)) for b in range(B)])
    o = o + np.asarray(bo, dtype=np.float32)[None, None, :]
    return o.astype(np.float32)


# revision 84
# speedup vs baseline: 1.0164x; 1.0164x over previous
"""Trainium2 Bass kernel: multi-head attention forward (B=2, S=2048, D=1024, H=16).

Sharding: 8 cores = data-parallel over batch (2) x tensor-parallel over heads
(4 head-groups of 4 heads).  Host sums the 4 partial outputs per batch and
adds the bias.

Per-core algorithm (all matmul operands bf16: 1 cycle/row at any width):
  qT/kT = w.T @ x.T        [256, S]   (per-head-group projections)
  v     = x @ wv           [S, 256]   (+ a ones column per head for the
                                       softmax denominators)
  per (query-block j, head h, key-tile ski):
      scoresT[sk, sq] = k_h @ q_h.T   (psum, 512-wide chunks, causal-trimmed)
      attnT = exp(scale * scoresT)    (ACT, psum -> SBUF bf16)
      causal mask on the diagonal tile (GPSIMD affine_select, fill 0)
  per query-tile qc (once its diagonal exp is done):
      ctx'[sq, 65] = sum_ski attnT_tile.T @ [v_h | 1]   (keys contracted on
          partitions, queries on output partitions: 65 cycles per key-tile
          instead of the 512 a [65, sq]-oriented PV pays)
      recip = 1/ctx'[:, 64]           (DVE approx; SBUF source only)
      ctx_n[sq, 64] = ctx' * recip    (per-partition scalar, bf16)
  per head-pair: PE-transpose ctx_n [sq,128] -> ctxT [128, sq] for the
      output projection (row-sharded wo), emitted per 128-row s-tile so the
      projection and its DMA overlap the remaining attention.
"""

import sys

sys.path.insert(0, "/opt/trn_rl_repo")

import numpy as np

B, S, D = 2, 2048, 1024
H = 16
DH = 64
HL = 4  # heads per core
NCORES = 8
W_SCALE = 32.0  # fp8 weight prescale so w ~ N(0, 0.02) clears e4m3 denormals

_PROGRAM_CACHE = {}


def build_program(S=S, D=D, HL=HL, DH=DH):
    from collections import deque

    import concourse.tile as tile
    from concourse import bacc, mybir

    f32 = mybir.dt.float32
    bf16 = mybir.dt.bfloat16
    f8 = mybir.dt.float8e4
    A = mybir.ActivationFunctionType
    Alu = mybir.AluOpType
    DR = mybir.MatmulPerfMode.DoubleRow

    KD = D // 128        # contraction chunks for the projections
    KD4 = D // 256       # fp8 DoubleRow contraction chunks (2 rows/partition)
    M = HL * DH          # per-core projected width (256)
    MQ = M // 128        # qT/kT partition tiles (2)
    ST = S // 128        # 128-row s tiles
    W = min(1024, S)     # query-block width
    NJ = S // W          # query blocks
    TPB = W // 128       # 128-tiles per query block
    NCH = S // 512       # 512-wide s chunks
    GA_N = min(2, NCH)   # s chunks covered by the k-outer warmup group
    scale = 1.0 / float(np.sqrt(DH))

    nc = bacc.Bacc("TRN2", target_bir_lowering=False, debug=False)
    xT = nc.dram_tensor("xT", (D, S), bf16, kind="ExternalInput").ap()
    # q/k path in fp8e4m3: weights host-prescaled by W_SCALE, undone in the
    # psum->sbuf copies; v/out stay bf16 (fp8 there fails the 2e-2 gate).
    # wq|wk are host-interleaved into one tensor so the DMA rows reach the
    # 512B no-penalty descriptor size.
    x8T = nc.dram_tensor("x8T", (D, S), f8, kind="ExternalInput").ap()
    wqk = nc.dram_tensor("wqk", (D, 2 * M), f8, kind="ExternalInput").ap()
    wv = nc.dram_tensor("wv", (D, M), bf16, kind="ExternalInput").ap()
    wo = nc.dram_tensor("wo", (M, D), bf16, kind="ExternalInput").ap()
    out = nc.dram_tensor("out", (S, D), bf16, kind="ExternalOutput").ap()

    with tile.TileContext(nc) as tc:
        with (
            tc.tile_pool(name="persist", bufs=1) as mpool,
            tc.tile_pool(name="ostage", bufs=3) as ostage,
            tc.tile_pool(name="rp", bufs=2) as rpool,
        ):
            wo_sb = mpool.tile([128, MQ, D], bf16, tag="wo")
            qT_sb = mpool.tile([128, MQ, S], bf16, tag="qT")
            kT_sb = mpool.tile([128, MQ, S], bf16, tag="kT")
            v_sb = mpool.tile([128, ST, HL, DH + 1], bf16, tag="v")
            ctx_sb = mpool.tile([128, MQ, S], bf16, tag="ctx")
            attn_sb = mpool.tile([128, 3, ST, W], bf16, tag="attn")
            ctxn_sb = mpool.tile([128, MQ, TPB, 128], bf16, tag="ctxn")
            ident = mpool.tile([128, 128], bf16, tag="id")
            xt = mpool.tile([128, KD, S], bf16, tag="xt")
            x8 = mpool.tile([128, KD4, 2, S], f8, tag="x8")
            wqk_sb = mpool.tile([128, KD4, 2, 2 * M], f8, tag="wqk")
            wv_sb = mpool.tile([128, KD, M], bf16, tag="wv")

            nc.gpsimd.memset(ident[:], 0.0)
            nc.gpsimd.affine_select(
                out=ident[:], in_=ident[:], compare_op=Alu.not_equal,
                fill=1.0, base=0, pattern=[[-1, 128]], channel_multiplier=1,
            )
            # ones columns for the PV denominator trick
            nc.gpsimd.memset(v_sb[:, :, :, DH], 1.0)

            # ---------------- loads (order gates the exp-stream start) ----
            wqk_r = wqk.rearrange("(c i p) m -> p c i m", p=128, i=2)
            wv_r = wv.rearrange("(k p) m -> p k m", p=128)
            xT_r = xT.rearrange("(k p) s -> p k s", p=128)
            x8_r = x8T.rearrange("(c i p) s -> p c i s", p=128, i=2)
            nhalf = 2 if NCH >= 4 else 1
            xh = S // nhalf
            nq = 4 if NCH >= 4 else 1
            xq = S // nq
            nc.sync.dma_start(wqk_sb[:], wqk_r[:])
            nc.sync.dma_start(x8[:, 0:KD4 // 2, :, 0:xh],
                              x8_r[:, 0:KD4 // 2, :, 0:xh])
            nc.sync.dma_start(x8[:, KD4 // 2:KD4, :, 0:xh],
                              x8_r[:, KD4 // 2:KD4, :, 0:xh])
            nc.sync.dma_start(xt[:, :, 0:xq], xT_r[:, :, 0:xq])
            nc.sync.dma_start(wv_sb[:], wv_r[:])
            if nq > 1:
                nc.sync.dma_start(xt[:, :, xq:2 * xq], xT_r[:, :, xq:2 * xq])
            if nhalf > 1:
                nc.sync.dma_start(x8[:, :, :, xh:S], x8_r[:, :, :, xh:S])
            if nq > 1:
                nc.sync.dma_start(xt[:, :, 2 * xq:3 * xq],
                                  xT_r[:, :, 2 * xq:3 * xq])
                nc.sync.dma_start(xt[:, :, 3 * xq:S], xT_r[:, :, 3 * xq:S])
            nc.sync.dma_start(wo_sb[:], wo.rearrange("(k p) d -> p k d", p=128))

            # ---------- warmup projections (c-outer over GA_N chunks) -----
            # q/k for all heads, s < GA_N*512: enough to start attention
            # (j=0, both passes) as soon as the fp8 first half of x lands.
            n_ga = (GA_N + 1) * MQ
            ga_cm = tc.tile_pool(name="gaps", bufs=n_ga, space="PSUM")
            gapool = ga_cm.__enter__()
            ga = [gapool.tile([128, 512], f32, tag="ga", name=f"ga{i}")
                  for i in range(n_ga)]
            # first-half-of-x chunks sweep across all tiles (DMA-paced),
            # then each tile finishes its second half and copies out
            # immediately, so the copies gating the first exps retire early
            descale = 1.0 / W_SCALE
            ga_tiles = []
            i = 0
            for m in range(MQ):
                for n in range(GA_N):
                    sl = slice(n * 512, (n + 1) * 512)
                    ga_tiles.append((ga[i], m * 128, qT_sb, m, sl, True))
                    i += 1
                    if n == 0:  # later kT chunks are deferred units
                        ga_tiles.append((ga[i], M + m * 128, kT_sb, m, sl,
                                         False))
                        i += 1
            for c in range(KD4 // 2):
                for t, col, dst, m, sl, is_q in ga_tiles:
                    nc.tensor.matmul(t[:], wqk_sb[:, c, :, col:col + 128],
                                     x8[:, c, :, sl], perf_mode=DR,
                                     start=(c == 0), stop=False)
            for t, col, dst, m, sl, is_q in ga_tiles:
                for c in range(KD4 // 2, KD4):
                    nc.tensor.matmul(t[:], wqk_sb[:, c, :, col:col + 128],
                                     x8[:, c, :, sl], perf_mode=DR,
                                     start=False, stop=(c == KD4 - 1))
                if is_q:
                    nc.vector.tensor_scalar(dst[:, m, sl], t[:],
                                            descale, None, Alu.mult)
                else:
                    nc.scalar.activation(dst[:, m, sl], t[:], A.Copy,
                                         scale=descale)
            ga_cm.__exit__(None, None, None)

            # --------------- attention-phase psum pools -------------------
            # entry order fixes bank placement: the scratch ring lands on the
            # banks whose warmup copies retire first, the scores ring next
            scr_cm = tc.tile_pool(name="scrps", bufs=2, space="PSUM")
            scrpool = scr_cm.__enter__()
            sc_cm = tc.tile_pool(name="scps", bufs=2, space="PSUM")
            spool = sc_cm.__enter__()
            ctx_cm = tc.tile_pool(name="ctxps", bufs=2, space="PSUM")
            cpool = ctx_cm.__enter__()

            # ------------- deferred projection units (one psum tile) ------
            def qk_unit(base, dst, m, n):
                def emit():
                    ps = scrpool.tile([128, 512], f32, tag="scr")
                    sl = slice(n * 512, (n + 1) * 512)
                    col = base + m * 128
                    for c in range(KD4):
                        nc.tensor.matmul(ps[:], wqk_sb[:, c, :, col:col + 128],
                                         x8[:, c, :, sl], perf_mode=DR,
                                         start=(c == 0), stop=(c == KD4 - 1))
                    nc.vector.tensor_scalar(dst[:, m, sl], ps[:],
                                            1.0 / W_SCALE, None, Alu.mult)
                return emit

            def v_unit(st):
                def emit():
                    ps = scrpool.tile([128, 512], f32, tag="scr")
                    for k in range(KD):
                        nc.tensor.matmul(ps[:, 0:M], xt[:, k, st * 128:(st + 1) * 128],
                                         wv_sb[:, k, :], start=(k == 0),
                                         stop=(k == KD - 1))
                    nc.vector.tensor_copy(
                        v_sb[:, st, :, 0:DH],
                        ps[:, 0:M].rearrange("p (h c) -> p h c", h=HL),
                    )
                return emit

            # early units feed j=0 (and j=1's first scores); late units are
            # only needed from j=1's deeper ski range on and fill j=1's
            # PE slack while ACT paces the exp stream
            # j=0 unit schedule keyed by (pass, ski), honoring deadlines:
            # kT n>=1 chunks before scores ski=4n, v(qc) before its burst at
            # step qc+1, qT chunks for j=1 anytime within pass B
            j0sched = {}

            def sched(p, ski, fn):
                j0sched.setdefault((p, ski), []).append(fn)

            for n in range(1, GA_N):
                for m in range(MQ):
                    sched(0, 0, qk_unit(M, kT_sb, m, n))
            # post-slot pops: v(qc) lands after step qc's emission, ahead of
            # its burst in step qc+1
            for qc in range(min(ST, TPB)):
                sched(0, qc, v_unit(qc))
            qn = [qk_unit(0, qT_sb, m, n)
                  for n in range(GA_N, NCH) for m in range(MQ)]
            for i, fn in enumerate(qn):
                sched(1, (2 * i) % TPB, fn)
            late = deque()
            if NJ > 1:
                # interleave the late v tiles with the late-needed kT
                # chunks: v(TPB+i) is read from burst step TPB+i on, kT
                # chunk n from scores step 4n on
                lv = [v_unit(st) for st in range(TPB, ST)]
                lk = [qk_unit(M, kT_sb, m, n)
                      for n in range(GA_N, NCH) for m in range(MQ)]
                while lv or lk:
                    if lv:
                        late.append(lv.pop(0))
                    if lk:
                        late.append(lk.pop(0))
            else:
                for st in range(TPB, ST):
                    sched(1, (2 * st) % TPB, v_unit(st))

            # ------------- output projection for one s-tile ---------------
            def emit_outproj_st(st, act_share):
                o_sb = ostage.tile([128, D], bf16, tag="o")
                for n in range(D // 512):
                    ops = scrpool.tile([128, 512], f32, tag="scr")
                    for p2 in range(MQ):
                        nc.tensor.matmul(
                            ops[:], ctx_sb[:, p2, st * 128:(st + 1) * 128],
                            wo_sb[:, p2, n * 512:(n + 1) * 512],
                            start=(p2 == 0), stop=(p2 == MQ - 1),
                        )
                    sl = slice(n * 512, (n + 1) * 512)
                    if n < D // 512 - act_share:
                        nc.vector.tensor_copy(o_sb[:, sl], ops[:])
                    else:
                        nc.scalar.activation(o_sb[:, sl], ops[:], A.Copy)
                nc.sync.dma_start(out[st * 128:(st + 1) * 128, :], o_sb[:])

            # ------------- attention for one (j, h) -----------------------
            outproj_q = deque()

            def emit_burst(j, h, qc, bank):
                """Flipped-PV burst for query-tile qc + softmax normalize;
                on the second head of a pair also transpose ctx to ctxT and
                queue the s-tile's output projection."""
                hm, po = h // 2, 64 * (h % 2)
                qt = j * TPB + qc
                ctx_ps = cpool.tile([128, DH + 1], f32, tag="ctx")
                for s2 in range(qt + 1):
                    nc.tensor.matmul(
                        ctx_ps[:],
                        attn_sb[:, bank, s2, qc * 128:(qc + 1) * 128],
                        v_sb[:, s2, h, :],
                        start=(s2 == 0), stop=(s2 == qt),
                    )
                dn = rpool.tile([128, 1], f32, tag="dn")
                rc = rpool.tile([128, 1], f32, tag="rc")
                nc.vector.tensor_copy(dn[:], ctx_ps[:, DH:DH + 1])
                # custom-DVE reciprocal must read SBUF (psum source returns
                # garbage on hardware); ~51 ULP is negligible
                nc.vector.reciprocal_approx_fast(out=rc[:], in_=dn[:])
                nc.vector.tensor_scalar(
                    ctxn_sb[:, hm, qc, po:po + DH], ctx_ps[:, 0:DH],
                    rc[:], None, Alu.mult,
                )
                if po:  # both heads of the pair done for this qc
                    tp = scrpool.tile([128, 512], f32, tag="scr")
                    tpb = tp[:, 0:64].bitcast(bf16)
                    nc.tensor.transpose(tpb, ctxn_sb[:, hm, qc, :], ident[:])
                    nc.vector.tensor_copy(
                        ctx_sb[:, hm, qt * 128:(qt + 1) * 128], tpb,
                    )
                    if hm == MQ - 1:
                        outproj_q.append(qt)

            def attention_pass(j, hs, pass_idx):
                """One ski sweep emitting two heads per step (one per
                attn bank/head-pair); bursts lag their diagonal exp by one
                step so the PE never queues a burst ahead of the next
                scores.  attn banks rotate mod 3 so a pass's first exps
                never WAR-wait on the previous pass's last bursts."""
                banks = {hh: (2 * pass_idx + i) % 3 for i, hh in enumerate(hs)}
                nski = TPB * (j + 1)
                for ski in range(nski):
                    # "pre" fillers (projection units) must precede the step
                    # (write-before-read in the PE queue); "post" fillers
                    # (output projections) go behind the step's scores so
                    # they never delay the exp stream
                    yield "pre", ski
                    ex0 = max(0, 128 * ski - j * W)
                    for h in hs:
                        hm, po = h // 2, 64 * (h % 2)
                        qrow = slice(po, po + DH)
                        sc = spool.tile([128, W], f32, tag="sc")
                        for n in range(ex0 // 512, W // 512):
                            lo = max(ex0, n * 512)
                            nc.tensor.matmul(
                                sc[:, lo:(n + 1) * 512],
                                kT_sb[qrow, hm, ski * 128:(ski + 1) * 128],
                                qT_sb[qrow, hm,
                                      j * W + lo: j * W + (n + 1) * 512],
                                start=True, stop=True,
                            )
                        at = attn_sb[:, banks[h], ski]
                        nc.scalar.activation(at[:, ex0:W], sc[:, ex0:W],
                                             A.Exp, scale=scale)
                        if 128 * ski >= j * W:  # diagonal-crossing tile
                            nc.gpsimd.affine_select(
                                out=at[:, ex0:ex0 + 128],
                                in_=at[:, ex0:ex0 + 128],
                                compare_op=Alu.is_ge, fill=0.0,
                                base=j * W + ex0 - 128 * ski,
                                pattern=[[1, 128]], channel_multiplier=-1,
                            )
                    if ski > j * TPB:
                        for h in hs:
                            emit_burst(j, h, ski - 1 - j * TPB, banks[h])
                    yield "post", ski
                for h in hs:
                    emit_burst(j, h, TPB - 1, banks[h])

            for j in range(NJ):
                if j > 0:
                    for fns in [j0sched.pop(k) for k in sorted(j0sched)]:
                        for fn in fns:
                            fn()
                nski_j = TPB * (j + 1)
                for p in range(2):
                    hs = (0, 2) if p == 0 else (1, 3)
                    if j > 0 and p == 1:
                        while late:
                            late.popleft()()
                    popped = False
                    for phase, ski in attention_pass(j, hs, 2 * j + p):
                        if phase == "pre":
                            popped = False
                        elif j == 0 and (p, ski) in j0sched:
                            popped = True
                            for fn in j0sched.pop((p, ski)):
                                fn()
                        elif j > 0 and p == 0 and late:
                            popped = True
                            late.popleft()()
                        elif not popped and outproj_q and (
                            (j == NJ - 1 and p == 1) or (j > 0 and ski % 3 == 2)
                        ):
                            tail = j == NJ - 1 and p == 1 and ski >= nski_j - 4
                            emit_outproj_st(outproj_q.popleft(),
                                            act_share=1 if tail else 0)
            while late:
                late.popleft()()
            while outproj_q:
                emit_outproj_st(outproj_q.popleft(), act_share=1)
            ctx_cm.__exit__(None, None, None)
            sc_cm.__exit__(None, None, None)
            scr_cm.__exit__(None, None, None)

    nc.compile()
    return nc


def _get_program():
    key = (S, D, HL, DH)
    if key not in _PROGRAM_CACHE:
        _PROGRAM_CACHE[key] = build_program(*key)
    return _PROGRAM_CACHE[key]


def prepare_core_inputs(xT, wq, wk, wv, wo):
    """fp32 per-core inputs (xT [D,S], wq/wk/wv [D,M], wo [M,D]) -> the
    device tensor map (bf16 + prescaled-fp8 q/k path)."""
    import ml_dtypes

    bf16 = ml_dtypes.bfloat16
    f8 = ml_dtypes.float8_e4m3
    xT = np.ascontiguousarray(xT, dtype=np.float32)
    wqk = np.concatenate([np.asarray(wq), np.asarray(wk)], axis=1) * W_SCALE
    return {
        "xT": xT.astype(bf16),
        "x8T": xT.astype(f8),
        "wqk": np.ascontiguousarray(wqk).astype(f8),
        "wv": np.ascontiguousarray(wv).astype(bf16),
        "wo": np.ascontiguousarray(wo).astype(bf16),
    }


def make_in_maps(x, Wq, Wk, Wv, Wo):
    x = np.asarray(x, dtype=np.float32)
    Wq = np.asarray(Wq, dtype=np.float32)
    Wk = np.asarray(Wk, dtype=np.float32)
    Wv = np.asarray(Wv, dtype=np.float32)
    Wo = np.asarray(Wo, dtype=np.float32)
    in_maps = []
    for c in range(NCORES):
        b, g = divmod(c, NCORES // B)
        sl = slice(HL * DH * g, HL * DH * (g + 1))
        in_maps.append(
            prepare_core_inputs(x[b].T, Wq[sl, :].T, Wk[sl, :].T,
                                Wv[sl, :].T, Wo[:, sl].T)
        )
    return in_maps


def kernel(x, Wq, Wk, Wv, Wo, bo):
    from concourse import bass2jax

    nc = _get_program()
    in_maps = make_in_maps(x, Wq, Wk, Wv, Wo)
    res = bass2jax.run_bass_via_pjrt(nc, in_maps, n_cores=NCORES)
    outs = [np.asarray(res[c]["out"], dtype=np.float32) for c in range(NCORES)]
    gpb = NCORES // B
    o = np.stack([sum(outs[b * gpb + g] for g in range(gpb)) for b in range(B)])
    o = o + np.asarray(bo, dtype=np.float32)[None, None, :]
    return o.astype(np.float32)


# revision 85
# speedup vs baseline: 1.0183x; 1.0019x over previous
"""Trainium2 Bass kernel: multi-head attention forward (B=2, S=2048, D=1024, H=16).

Sharding: 8 cores = data-parallel over batch (2) x tensor-parallel over heads
(4 head-groups of 4 heads).  Host sums the 4 partial outputs per batch and
adds the bias.

Per-core algorithm (all matmul operands bf16: 1 cycle/row at any width):
  qT/kT = w.T @ x.T        [256, S]   (per-head-group projections)
  v     = x @ wv           [S, 256]   (+ a ones column per head for the
                                       softmax denominators)
  per (query-block j, head h, key-tile ski):
      scoresT[sk, sq] = k_h @ q_h.T   (psum, 512-wide chunks, causal-trimmed)
      attnT = exp(scale * scoresT)    (ACT, psum -> SBUF bf16)
      causal mask on the diagonal tile (GPSIMD affine_select, fill 0)
  per query-tile qc (once its diagonal exp is done):
      ctx'[sq, 65] = sum_ski attnT_tile.T @ [v_h | 1]   (keys contracted on
          partitions, queries on output partitions: 65 cycles per key-tile
          instead of the 512 a [65, sq]-oriented PV pays)
      recip = 1/ctx'[:, 64]           (DVE approx; SBUF source only)
      ctx_n[sq, 64] = ctx' * recip    (per-partition scalar, bf16)
  per head-pair: PE-transpose ctx_n [sq,128] -> ctxT [128, sq] for the
      output projection (row-sharded wo), emitted per 128-row s-tile so the
      projection and its DMA overlap the remaining attention.
"""

import sys

sys.path.insert(0, "/opt/trn_rl_repo")

import numpy as np

B, S, D = 2, 2048, 1024
H = 16
DH = 64
HL = 4  # heads per core
NCORES = 8
W_SCALE = 32.0  # fp8 weight prescale so w ~ N(0, 0.02) clears e4m3 denormals

_PROGRAM_CACHE = {}


def build_program(S=S, D=D, HL=HL, DH=DH):
    from collections import deque

    import concourse.tile as tile
    from concourse import bacc, mybir

    f32 = mybir.dt.float32
    bf16 = mybir.dt.bfloat16
    f8 = mybir.dt.float8e4
    A = mybir.ActivationFunctionType
    Alu = mybir.AluOpType
    DR = mybir.MatmulPerfMode.DoubleRow

    KD = D // 128        # contraction chunks for the projections
    KD4 = D // 256       # fp8 DoubleRow contraction chunks (2 rows/partition)
    M = HL * DH          # per-core projected width (256)
    MQ = M // 128        # qT/kT partition tiles (2)
    ST = S // 128        # 128-row s tiles
    W = min(1024, S)     # query-block width
    NJ = S // W          # query blocks
    TPB = W // 128       # 128-tiles per query block
    NCH = S // 512       # 512-wide s chunks
    GA_N = min(2, NCH)   # s chunks covered by the k-outer warmup group
    scale = 1.0 / float(np.sqrt(DH))

    nc = bacc.Bacc("TRN2", target_bir_lowering=False, debug=False)
    xT = nc.dram_tensor("xT", (D, S), bf16, kind="ExternalInput").ap()
    # q/k path in fp8e4m3: weights host-prescaled by W_SCALE, undone in the
    # psum->sbuf copies; v/out stay bf16 (fp8 there fails the 2e-2 gate).
    # wq|wk are host-interleaved into one tensor so the DMA rows reach the
    # 512B no-penalty descriptor size.
    x8T = nc.dram_tensor("x8T", (D, S), f8, kind="ExternalInput").ap()
    wqk = nc.dram_tensor("wqk", (D, 2 * M), f8, kind="ExternalInput").ap()
    wv = nc.dram_tensor("wv", (D, M), bf16, kind="ExternalInput").ap()
    wo = nc.dram_tensor("wo", (M, D), bf16, kind="ExternalInput").ap()
    out = nc.dram_tensor("out", (S, D), bf16, kind="ExternalOutput").ap()

    with tile.TileContext(nc) as tc:
        with (
            tc.tile_pool(name="persist", bufs=1) as mpool,
            tc.tile_pool(name="ostage", bufs=3) as ostage,
            tc.tile_pool(name="rp", bufs=2) as rpool,
        ):
            wo_sb = mpool.tile([128, MQ, D], bf16, tag="wo")
            qT_sb = mpool.tile([128, MQ, S], bf16, tag="qT")
            kT_sb = mpool.tile([128, MQ, S], bf16, tag="kT")
            v_sb = mpool.tile([128, ST, HL, DH + 1], bf16, tag="v")
            ctx_sb = mpool.tile([128, MQ, S], bf16, tag="ctx")
            attn_sb = mpool.tile([128, 3, ST, W], bf16, tag="attn")
            ctxn_sb = mpool.tile([128, MQ, TPB, 128], bf16, tag="ctxn")
            ident = mpool.tile([128, 128], bf16, tag="id")
            xt = mpool.tile([128, KD, S], bf16, tag="xt")
            x8 = mpool.tile([128, KD4, 2, S], f8, tag="x8")
            wqk_sb = mpool.tile([128, KD4, 2, 2 * M], f8, tag="wqk")
            wv_sb = mpool.tile([128, KD, M], bf16, tag="wv")

            nc.gpsimd.memset(ident[:], 0.0)
            nc.gpsimd.affine_select(
                out=ident[:], in_=ident[:], compare_op=Alu.not_equal,
                fill=1.0, base=0, pattern=[[-1, 128]], channel_multiplier=1,
            )
            # ones columns for the PV denominator trick
            nc.gpsimd.memset(v_sb[:, :, :, DH], 1.0)

            # ---------------- loads (order gates the exp-stream start) ----
            wqk_r = wqk.rearrange("(c i p) m -> p c i m", p=128, i=2)
            wv_r = wv.rearrange("(k p) m -> p k m", p=128)
            xT_r = xT.rearrange("(k p) s -> p k s", p=128)
            x8_r = x8T.rearrange("(c i p) s -> p c i s", p=128, i=2)
            nhalf = 2 if NCH >= 4 else 1
            xh = S // nhalf
            nq = 4 if NCH >= 4 else 1
            xq = S // nq
            nc.sync.dma_start(wqk_sb[:], wqk_r[:])
            nc.sync.dma_start(x8[:, 0:KD4 // 2, :, 0:xh],
                              x8_r[:, 0:KD4 // 2, :, 0:xh])
            nc.sync.dma_start(x8[:, KD4 // 2:KD4, :, 0:xh],
                              x8_r[:, KD4 // 2:KD4, :, 0:xh])
            nc.sync.dma_start(xt[:, :, 0:xq], xT_r[:, :, 0:xq])
            nc.sync.dma_start(wv_sb[:], wv_r[:])
            if nq > 1:
                nc.sync.dma_start(xt[:, :, xq:2 * xq], xT_r[:, :, xq:2 * xq])
            if nhalf > 1:
                nc.sync.dma_start(x8[:, :, :, xh:S], x8_r[:, :, :, xh:S])
            if nq > 1:
                nc.sync.dma_start(xt[:, :, 2 * xq:3 * xq],
                                  xT_r[:, :, 2 * xq:3 * xq])
                nc.sync.dma_start(xt[:, :, 3 * xq:S], xT_r[:, :, 3 * xq:S])
            nc.sync.dma_start(wo_sb[:], wo.rearrange("(k p) d -> p k d", p=128))

            # ---------- warmup projections (c-outer over GA_N chunks) -----
            # q/k for all heads, s < GA_N*512: enough to start attention
            # (j=0, both passes) as soon as the fp8 first half of x lands.
            n_ga = (GA_N + 1) * MQ
            ga_cm = tc.tile_pool(name="gaps", bufs=n_ga, space="PSUM")
            gapool = ga_cm.__enter__()
            ga = [gapool.tile([128, 512], f32, tag="ga", name=f"ga{i}")
                  for i in range(n_ga)]
            # first-half-of-x chunks sweep across all tiles (DMA-paced),
            # then each tile finishes its second half and copies out
            # immediately, so the copies gating the first exps retire early
            descale = 1.0 / W_SCALE
            ga_tiles = []
            i = 0
            for m in range(MQ):
                for n in range(GA_N):
                    sl = slice(n * 512, (n + 1) * 512)
                    ga_tiles.append((ga[i], m * 128, qT_sb, m, sl, True))
                    i += 1
                    if n == 0:  # later kT chunks are deferred units
                        ga_tiles.append((ga[i], M + m * 128, kT_sb, m, sl,
                                         False))
                        i += 1
            for c in range(KD4 // 2):
                for t, col, dst, m, sl, is_q in ga_tiles:
                    nc.tensor.matmul(t[:], wqk_sb[:, c, :, col:col + 128],
                                     x8[:, c, :, sl], perf_mode=DR,
                                     start=(c == 0), stop=False)
            for t, col, dst, m, sl, is_q in ga_tiles:
                for c in range(KD4 // 2, KD4):
                    nc.tensor.matmul(t[:], wqk_sb[:, c, :, col:col + 128],
                                     x8[:, c, :, sl], perf_mode=DR,
                                     start=False, stop=(c == KD4 - 1))
                if is_q:
                    nc.vector.tensor_scalar(dst[:, m, sl], t[:],
                                            descale, None, Alu.mult)
                else:
                    nc.scalar.activation(dst[:, m, sl], t[:], A.Copy,
                                         scale=descale)
            ga_cm.__exit__(None, None, None)

            # --------------- attention-phase psum pools -------------------
            # entry order fixes bank placement: the scratch ring lands on the
            # banks whose warmup copies retire first, the scores ring next
            scr_cm = tc.tile_pool(name="scrps", bufs=2, space="PSUM")
            scrpool = scr_cm.__enter__()
            sc_cm = tc.tile_pool(name="scps", bufs=2, space="PSUM")
            spool = sc_cm.__enter__()
            ctx_cm = tc.tile_pool(name="ctxps", bufs=2, space="PSUM")
            cpool = ctx_cm.__enter__()

            # ------------- deferred projection units (one psum tile) ------
            def qk_unit(base, dst, m, n):
                def emit():
                    ps = scrpool.tile([128, 512], f32, tag="scr")
                    sl = slice(n * 512, (n + 1) * 512)
                    col = base + m * 128
                    for c in range(KD4):
                        nc.tensor.matmul(ps[:], wqk_sb[:, c, :, col:col + 128],
                                         x8[:, c, :, sl], perf_mode=DR,
                                         start=(c == 0), stop=(c == KD4 - 1))
                    nc.vector.tensor_scalar(dst[:, m, sl], ps[:],
                                            1.0 / W_SCALE, None, Alu.mult)
                return emit

            def v_unit(st):
                def emit():
                    ps = scrpool.tile([128, 512], f32, tag="scr")
                    for k in range(KD):
                        nc.tensor.matmul(ps[:, 0:M], xt[:, k, st * 128:(st + 1) * 128],
                                         wv_sb[:, k, :], start=(k == 0),
                                         stop=(k == KD - 1))
                    nc.vector.tensor_copy(
                        v_sb[:, st, :, 0:DH],
                        ps[:, 0:M].rearrange("p (h c) -> p h c", h=HL),
                    )
                return emit

            # early units feed j=0 (and j=1's first scores); late units are
            # only needed from j=1's deeper ski range on and fill j=1's
            # PE slack while ACT paces the exp stream
            # j=0 unit schedule keyed by (pass, ski), honoring deadlines:
            # kT n>=1 chunks before scores ski=4n, v(qc) before its burst at
            # step qc+1, qT chunks for j=1 anytime within pass B
            j0sched = {}

            def sched(p, ski, fn):
                j0sched.setdefault((p, ski), []).append(fn)

            for n in range(1, GA_N):
                for m in range(MQ):
                    sched(0, 0, qk_unit(M, kT_sb, m, n))
            # post-slot pops: v(qc) lands after step qc's emission, ahead of
            # its burst in step qc+1
            for qc in range(min(ST, TPB)):
                sched(0, qc, v_unit(qc))
            qn = [qk_unit(0, qT_sb, m, n)
                  for n in range(GA_N, NCH) for m in range(MQ)]
            for i, fn in enumerate(qn):
                sched(1, (2 * i) % TPB, fn)
            late = deque()
            if NJ > 1:
                # interleave the late v tiles with the late-needed kT
                # chunks: v(TPB+i) is read from burst step TPB+i on, kT
                # chunk n from scores step 4n on
                lv = [v_unit(st) for st in range(TPB, ST)]
                lk = [qk_unit(M, kT_sb, m, n)
                      for n in range(GA_N, NCH) for m in range(MQ)]
                while lv or lk:
                    if lv:
                        late.append(lv.pop(0))
                    if lk:
                        late.append(lk.pop(0))
            else:
                for st in range(TPB, ST):
                    sched(1, (2 * st) % TPB, v_unit(st))

            # ------------- output projection for one s-tile ---------------
            def emit_outproj_st(st, act_share):
                o_sb = ostage.tile([128, D], bf16, tag="o")
                for n in range(D // 512):
                    ops = scrpool.tile([128, 512], f32, tag="scr")
                    for p2 in range(MQ):
                        nc.tensor.matmul(
                            ops[:], ctx_sb[:, p2, st * 128:(st + 1) * 128],
                            wo_sb[:, p2, n * 512:(n + 1) * 512],
                            start=(p2 == 0), stop=(p2 == MQ - 1),
                        )
                    sl = slice(n * 512, (n + 1) * 512)
                    if n < D // 512 - act_share:
                        nc.vector.tensor_copy(o_sb[:, sl], ops[:])
                    else:
                        nc.scalar.activation(o_sb[:, sl], ops[:], A.Copy)
                nc.sync.dma_start(out[st * 128:(st + 1) * 128, :], o_sb[:])

            # ------------- attention for one (j, h) -----------------------
            outproj_q = deque()

            def emit_burst(j, h, qc, bank):
                """Flipped-PV burst for query-tile qc + softmax normalize;
                on the second head of a pair also transpose ctx to ctxT and
                queue the s-tile's output projection."""
                hm, po = h // 2, 64 * (h % 2)
                qt = j * TPB + qc
                ctx_ps = cpool.tile([128, DH + 1], f32, tag="ctx")
                for s2 in range(qt + 1):
                    nc.tensor.matmul(
                        ctx_ps[:],
                        attn_sb[:, bank, s2, qc * 128:(qc + 1) * 128],
                        v_sb[:, s2, h, :],
                        start=(s2 == 0), stop=(s2 == qt),
                    )
                dn = rpool.tile([128, 1], f32, tag="dn")
                nc.vector.tensor_copy(dn[:], ctx_ps[:, DH:DH + 1])
                # per-partition divide; the scalar operand must be SBUF,
                # hence the denominator copy
                nc.vector.tensor_scalar(
                    ctxn_sb[:, hm, qc, po:po + DH], ctx_ps[:, 0:DH],
                    dn[:], None, Alu.divide,
                )
                if po:  # both heads of the pair done for this qc
                    tp = scrpool.tile([128, 512], f32, tag="scr")
                    tpb = tp[:, 0:64].bitcast(bf16)
                    nc.tensor.transpose(tpb, ctxn_sb[:, hm, qc, :], ident[:])
                    nc.vector.tensor_copy(
                        ctx_sb[:, hm, qt * 128:(qt + 1) * 128], tpb,
                    )
                    if hm == MQ - 1:
                        outproj_q.append(qt)

            def attention_pass(j, hs, pass_idx):
                """One ski sweep emitting two heads per step (one per
                attn bank/head-pair); bursts lag their diagonal exp by one
                step so the PE never queues a burst ahead of the next
                scores.  attn banks rotate mod 3 so a pass's first exps
                never WAR-wait on the previous pass's last bursts."""
                banks = {hh: (2 * pass_idx + i) % 3 for i, hh in enumerate(hs)}
                nski = TPB * (j + 1)
                for ski in range(nski):
                    # "pre" fillers (projection units) must precede the step
                    # (write-before-read in the PE queue); "post" fillers
                    # (output projections) go behind the step's scores so
                    # they never delay the exp stream
                    yield "pre", ski
                    ex0 = max(0, 128 * ski - j * W)
                    for h in hs:
                        hm, po = h // 2, 64 * (h % 2)
                        qrow = slice(po, po + DH)
                        sc = spool.tile([128, W], f32, tag="sc")
                        for n in range(ex0 // 512, W // 512):
                            lo = max(ex0, n * 512)
                            nc.tensor.matmul(
                                sc[:, lo:(n + 1) * 512],
                                kT_sb[qrow, hm, ski * 128:(ski + 1) * 128],
                                qT_sb[qrow, hm,
                                      j * W + lo: j * W + (n + 1) * 512],
                                start=True, stop=True,
                            )
                        at = attn_sb[:, banks[h], ski]
                        nc.scalar.activation(at[:, ex0:W], sc[:, ex0:W],
                                             A.Exp, scale=scale)
                        if 128 * ski >= j * W:  # diagonal-crossing tile
                            nc.gpsimd.affine_select(
                                out=at[:, ex0:ex0 + 128],
                                in_=at[:, ex0:ex0 + 128],
                                compare_op=Alu.is_ge, fill=0.0,
                                base=j * W + ex0 - 128 * ski,
                                pattern=[[1, 128]], channel_multiplier=-1,
                            )
                    if ski > j * TPB:
                        for h in hs:
                            emit_burst(j, h, ski - 1 - j * TPB, banks[h])
                    yield "post", ski
                for h in hs:
                    emit_burst(j, h, TPB - 1, banks[h])

            for j in range(NJ):
                if j > 0:
                    for fns in [j0sched.pop(k) for k in sorted(j0sched)]:
                        for fn in fns:
                            fn()
                nski_j = TPB * (j + 1)
                for p in range(2):
                    hs = (0, 2) if p == 0 else (1, 3)
                    if j > 0 and p == 1:
                        while late:
                            late.popleft()()
                    popped = False
                    for phase, ski in attention_pass(j, hs, 2 * j + p):
                        if phase == "pre":
                            popped = False
                        elif j == 0 and (p, ski) in j0sched:
                            popped = True
                            for fn in j0sched.pop((p, ski)):
                                fn()
                        elif j > 0 and p == 0 and late:
                            popped = True
                            late.popleft()()
                        elif not popped and outproj_q and (
                            (j == NJ - 1 and p == 1) or (j > 0 and ski % 3 == 2)
                        ):
                            tail = j == NJ - 1 and p == 1 and ski >= nski_j - 4
                            emit_outproj_st(outproj_q.popleft(),
                                            act_share=1 if tail else 0)
            while late:
                late.popleft()()
            while outproj_q:
                emit_outproj_st(outproj_q.popleft(), act_share=1)
            ctx_cm.__exit__(None, None, None)
            sc_cm.__exit__(None, None, None)
            scr_cm.__exit__(None, None, None)

    nc.compile()
    return nc


def _get_program():
    key = (S, D, HL, DH)
    if key not in _PROGRAM_CACHE:
        _PROGRAM_CACHE[key] = build_program(*key)
    return _PROGRAM_CACHE[key]


def prepare_core_inputs(xT, wq, wk, wv, wo):
    """fp32 per-core inputs (xT [D,S], wq/wk/wv [D,M], wo [M,D]) -> the
    device tensor map (bf16 + prescaled-fp8 q/k path)."""
    import ml_dtypes

    bf16 = ml_dtypes.bfloat16
    f8 = ml_dtypes.float8_e4m3
    xT = np.ascontiguousarray(xT, dtype=np.float32)
    wqk = np.concatenate([np.asarray(wq), np.asarray(wk)], axis=1) * W_SCALE
    return {
        "xT": xT.astype(bf16),
        "x8T": xT.astype(f8),
        "wqk": np.ascontiguousarray(wqk).astype(f8),
        "wv": np.ascontiguousarray(wv).astype(bf16),
        "wo": np.ascontiguousarray(wo).astype(bf16),
    }


def make_in_maps(x, Wq, Wk, Wv, Wo):
    x = np.asarray(x, dtype=np.float32)
    Wq = np.asarray(Wq, dtype=np.float32)
    Wk = np.asarray(Wk, dtype=np.float32)
    Wv = np.asarray(Wv, dtype=np.float32)
    Wo = np.asarray(Wo, dtype=np.float32)
    in_maps = []
    for c in range(NCORES):
        b, g = divmod(c, NCORES // B)
        sl = slice(HL * DH * g, HL * DH * (g + 1))
        in_maps.append(
            prepare_core_inputs(x[b].T, Wq[sl, :].T, Wk[sl, :].T,
                                Wv[sl, :].T, Wo[:, sl].T)
        )
    return in_maps


def kernel(x, Wq, Wk, Wv, Wo, bo):
    from concourse import bass2jax

    nc = _get_program()
    in_maps = make_in_maps(x, Wq, Wk, Wv, Wo)
    res = bass2jax.run_bass_via_pjrt(nc, in_maps, n_cores=NCORES)
    outs = [np.asarray(res[c]["out"], dtype=np.float32) for c in range(NCORES)]
    gpb = NCORES // B
    o = np.stack([sum(outs[b * gpb + g] for g in range(gpb)) for b in range(B)])
    o = o + np.asarray(bo, dtype=np.float32)[None, None, :]
    return o.astype(np.float32)


# revision 95
# speedup vs baseline: 1.0569x; 1.0380x over previous
"""Trainium2 Bass kernel: multi-head attention forward (B=2, S=2048, D=1024, H=16).

Sharding: 8 cores = data-parallel over batch (2) x tensor-parallel over heads
(4 head-groups of 4 heads).  Host sums the 4 partial outputs per batch and
adds the bias.

Per-core algorithm (all matmul operands bf16: 1 cycle/row at any width):
  qT/kT = w.T @ x.T        [256, S]   (per-head-group projections)
  v     = x @ wv           [S, 256]   (+ a ones column per head for the
                                       softmax denominators)
  per (query-block j, head h, key-tile ski):
      scoresT[sk, sq] = k_h @ q_h.T   (psum, 512-wide chunks, causal-trimmed)
      attnT = exp(scale * scoresT)    (ACT, psum -> SBUF bf16)
      causal mask on the diagonal tile (GPSIMD affine_select, fill 0)
  per query-tile qc (once its diagonal exp is done):
      ctx'[sq, 65] = sum_ski attnT_tile.T @ [v_h | 1]   (keys contracted on
          partitions, queries on output partitions: 65 cycles per key-tile
          instead of the 512 a [65, sq]-oriented PV pays)
      recip = 1/ctx'[:, 64]           (DVE approx; SBUF source only)
      ctx_n[sq, 64] = ctx' * recip    (per-partition scalar, bf16)
  per head-pair: PE-transpose ctx_n [sq,128] -> ctxT [128, sq] for the
      output projection (row-sharded wo), emitted per 128-row s-tile so the
      projection and its DMA overlap the remaining attention.
"""

import sys

sys.path.insert(0, "/opt/trn_rl_repo")

import numpy as np

B, S, D = 2, 2048, 1024
H = 16
DH = 64
HL = 4  # heads per core
NCORES = 8
W_SCALE = 32.0  # fp8 weight prescale so w ~ N(0, 0.02) clears e4m3 denormals

_PROGRAM_CACHE = {}


def build_program(S=S, D=D, HL=HL, DH=DH):
    from collections import deque

    import concourse.tile as tile
    from concourse import bacc, mybir

    f32 = mybir.dt.float32
    bf16 = mybir.dt.bfloat16
    f8 = mybir.dt.float8e4
    A = mybir.ActivationFunctionType
    Alu = mybir.AluOpType
    DR = mybir.MatmulPerfMode.DoubleRow

    KD = D // 128        # contraction chunks for the projections
    KD4 = D // 256       # fp8 DoubleRow contraction chunks (2 rows/partition)
    M = HL * DH          # per-core projected width (256)
    MQ = M // 128        # qT/kT partition tiles (2)
    ST = S // 128        # 128-row s tiles
    W = min(1024, S)     # query-block width
    NJ = S // W          # query blocks
    TPB = W // 128       # 128-tiles per query block
    NCH = S // 512       # 512-wide s chunks
    GA_N = min(2, NCH)   # s chunks covered by the k-outer warmup group
    N_WARMUP = 12 if NCH >= 4 else 0
    scale = 1.0 / float(np.sqrt(DH))

    nc = bacc.Bacc("TRN2", target_bir_lowering=False, debug=False)
    xT = nc.dram_tensor("xT", (D, S), bf16, kind="ExternalInput").ap()
    # q/k path in fp8e4m3: weights host-prescaled by W_SCALE, undone in the
    # psum->sbuf copies; v/out stay bf16 (fp8 there fails the 2e-2 gate).
    # wq|wk are host-interleaved into one tensor so the DMA rows reach the
    # 512B no-penalty descriptor size.
    x8T = nc.dram_tensor("x8T", (D, S), f8, kind="ExternalInput").ap()
    wqk = nc.dram_tensor("wqk", (D, 2 * M), f8, kind="ExternalInput").ap()
    wv = nc.dram_tensor("wv", (D, M), bf16, kind="ExternalInput").ap()
    wo = nc.dram_tensor("wo", (M, D), bf16, kind="ExternalInput").ap()
    out = nc.dram_tensor("out", (S, D), bf16, kind="ExternalOutput").ap()

    with tile.TileContext(nc) as tc:
        with (
            tc.tile_pool(name="persist", bufs=1) as mpool,
            tc.tile_pool(name="ostage", bufs=6) as ostage,
            tc.tile_pool(name="rp", bufs=2) as rpool,
        ):
            wo_sb = mpool.tile([128, MQ, D], bf16, tag="wo")
            qT_sb = mpool.tile([128, MQ, S], bf16, tag="qT")
            kT_sb = mpool.tile([128, MQ, S], bf16, tag="kT")
            v_sb = mpool.tile([128, ST, HL, DH + 1], bf16, tag="v")
            ctx_sb = mpool.tile([128, MQ, S], bf16, tag="ctx")
            # bank dim innermost-but-one so a paired two-bank activation
            # AP's bank stride (W elems) fits the 16-bit ISA stride field
            attn_sb = mpool.tile([128, ST, 3, W], bf16, tag="attn")
            ctxn_sb = mpool.tile([128, MQ, TPB, 128], bf16, tag="ctxn")
            ident = mpool.tile([128, 128], bf16, tag="id")
            xt = mpool.tile([128, KD, S], bf16, tag="xt")
            x8 = mpool.tile([128, KD4, 2, S], f8, tag="x8")
            wqk_sb = mpool.tile([128, KD4, 2, 2 * M], f8, tag="wqk")
            wv_sb = mpool.tile([128, KD, M], bf16, tag="wv")

            nc.gpsimd.memset(ident[:], 0.0)
            nc.gpsimd.affine_select(
                out=ident[:], in_=ident[:], compare_op=Alu.not_equal,
                fill=1.0, base=0, pattern=[[-1, 128]], channel_multiplier=1,
            )
            # ones columns for the PV denominator trick
            nc.gpsimd.memset(v_sb[:, :, :, DH], 1.0)

            # ---------------- loads (order gates the exp-stream start) ----
            wqk_r = wqk.rearrange("(c i p) m -> p c i m", p=128, i=2)
            wv_r = wv.rearrange("(k p) m -> p k m", p=128)
            xT_r = xT.rearrange("(k p) s -> p k s", p=128)
            x8_r = x8T.rearrange("(c i p) s -> p c i s", p=128, i=2)
            nhalf = 2 if NCH >= 4 else 1
            xh = S // nhalf
            nq = 4 if NCH >= 4 else 1
            xq = S // nq
            nc.sync.dma_start(wqk_sb[:], wqk_r[:])
            nc.sync.dma_start(x8[:, 0:KD4 // 2, :, 0:xh],
                              x8_r[:, 0:KD4 // 2, :, 0:xh])
            nc.sync.dma_start(x8[:, KD4 // 2:KD4, :, 0:xh],
                              x8_r[:, KD4 // 2:KD4, :, 0:xh])
            nc.sync.dma_start(xt[:, :, 0:xq], xT_r[:, :, 0:xq])
            nc.sync.dma_start(wv_sb[:], wv_r[:])
            if nq > 1:
                nc.sync.dma_start(xt[:, :, xq:2 * xq], xT_r[:, :, xq:2 * xq])
            if nhalf > 1:
                nc.sync.dma_start(x8[:, :, :, xh:S], x8_r[:, :, :, xh:S])
            if nq > 1:
                nc.sync.dma_start(xt[:, :, 2 * xq:3 * xq],
                                  xT_r[:, :, 2 * xq:3 * xq])
                nc.sync.dma_start(xt[:, :, 3 * xq:S], xT_r[:, :, 3 * xq:S])
            nc.sync.dma_start(wo_sb[:], wo.rearrange("(k p) d -> p k d", p=128))

            # ---------- warmup projections (c-outer over GA_N chunks) -----
            # q/k for all heads, s < GA_N*512: enough to start attention
            # (j=0, both passes) as soon as the fp8 first half of x lands.
            n_ga = (GA_N + 1) * MQ
            wu_cm = tc.tile_pool(name="wups", bufs=1, space="PSUM")
            wupool = wu_cm.__enter__()
            wu = wupool.tile([128, 128], bf16, tag="wu")
            # dummy transposes keep the PE continuously busy through the
            # input DMA so the warmup projections run at full p-state
            for _ in range(N_WARMUP):
                nc.tensor.transpose(wu[:], ident[:], ident[:])
            wu_cm.__exit__(None, None, None)
            ga_cm = tc.tile_pool(name="gaps", bufs=n_ga, space="PSUM")
            gapool = ga_cm.__enter__()
            ga = [gapool.tile([128, 512], f32, tag="ga", name=f"ga{i}")
                  for i in range(n_ga)]
            # first-half-of-x chunks sweep across all tiles (DMA-paced),
            # then each tile finishes its second half and copies out
            # immediately, so the copies gating the first exps retire early
            descale = 1.0 / W_SCALE
            ga_tiles = []
            i = 0
            for m in range(MQ):
                for n in range(GA_N):
                    sl = slice(n * 512, (n + 1) * 512)
                    ga_tiles.append((ga[i], m * 128, qT_sb, m, sl, True))
                    i += 1
                    if n == 0:  # later kT chunks are deferred units
                        ga_tiles.append((ga[i], M + m * 128, kT_sb, m, sl,
                                         False))
                        i += 1
            for c in range(KD4 // 2):
                for t, col, dst, m, sl, is_q in ga_tiles:
                    nc.tensor.matmul(t[:], wqk_sb[:, c, :, col:col + 128],
                                     x8[:, c, :, sl], perf_mode=DR,
                                     start=(c == 0), stop=False)
            for t, col, dst, m, sl, is_q in ga_tiles:
                for c in range(KD4 // 2, KD4):
                    nc.tensor.matmul(t[:], wqk_sb[:, c, :, col:col + 128],
                                     x8[:, c, :, sl], perf_mode=DR,
                                     start=False, stop=(c == KD4 - 1))
                if is_q:
                    nc.vector.tensor_scalar(dst[:, m, sl], t[:],
                                            descale, None, Alu.mult)
                else:
                    nc.scalar.activation(dst[:, m, sl], t[:], A.Copy,
                                         scale=descale)
            ga_cm.__exit__(None, None, None)

            # --------------- attention-phase psum pools -------------------
            # entry order fixes bank placement: the scratch ring lands on the
            # banks whose warmup copies retire first, the scores ring next
            scr_cm = tc.tile_pool(name="scrps", bufs=2, space="PSUM")
            scrpool = scr_cm.__enter__()
            sc_cm = tc.tile_pool(name="scps", bufs=2, space="PSUM")
            spool = sc_cm.__enter__()
            ctx_cm = tc.tile_pool(name="ctxps", bufs=2, space="PSUM")
            cpool = ctx_cm.__enter__()

            # ------------- deferred projection units (one psum tile) ------
            def qk_unit(base, dst, m, n):
                def emit():
                    ps = scrpool.tile([128, 512], f32, tag="scr")
                    sl = slice(n * 512, (n + 1) * 512)
                    col = base + m * 128
                    for c in range(KD4):
                        nc.tensor.matmul(ps[:], wqk_sb[:, c, :, col:col + 128],
                                         x8[:, c, :, sl], perf_mode=DR,
                                         start=(c == 0), stop=(c == KD4 - 1))
                    nc.vector.tensor_scalar(dst[:, m, sl], ps[:],
                                            1.0 / W_SCALE, None, Alu.mult)
                return emit

            def v_unit(st):
                def emit():
                    ps = scrpool.tile([128, 512], f32, tag="scr")
                    for k in range(KD):
                        nc.tensor.matmul(ps[:, 0:M], xt[:, k, st * 128:(st + 1) * 128],
                                         wv_sb[:, k, :], start=(k == 0),
                                         stop=(k == KD - 1))
                    nc.vector.tensor_copy(
                        v_sb[:, st, :, 0:DH],
                        ps[:, 0:M].rearrange("p (h c) -> p h c", h=HL),
                    )
                return emit

            # early units feed j=0 (and j=1's first scores); late units are
            # only needed from j=1's deeper ski range on and fill j=1's
            # PE slack while ACT paces the exp stream
            # j=0 unit schedule keyed by (pass, ski), honoring deadlines:
            # kT n>=1 chunks before scores ski=4n, v(qc) before its burst at
            # step qc+1, qT chunks for j=1 anytime within pass B
            j0sched = {}

            def sched(p, ski, fn):
                j0sched.setdefault((p, ski), []).append(fn)

            for n in range(1, GA_N):
                for m in range(MQ):
                    sched(0, 0, qk_unit(M, kT_sb, m, n))
            # post-slot pops: v(qc) lands after step qc's emission, ahead
            # of its burst in step qc+1 (the last tile's burst can retire
            # in-loop, so its v leads by one)
            for qc in range(min(ST, TPB)):
                sched(0, min(qc, TPB - 2), v_unit(qc))
            qn = [qk_unit(0, qT_sb, m, n)
                  for n in range(GA_N, NCH) for m in range(MQ)]
            for i, fn in enumerate(qn):
                sched(1, (2 * i) % TPB, fn)
            late = deque()
            if NJ > 1:
                # interleave the late v tiles with the late-needed kT
                # chunks: v(TPB+i) is read from burst step TPB+i on, kT
                # chunk n from scores step 4n on
                lv = [v_unit(st) for st in range(TPB, ST)]
                lk = [qk_unit(M, kT_sb, m, n)
                      for n in range(GA_N, NCH) for m in range(MQ)]
                while lv or lk:
                    if lv:
                        late.append(lv.pop(0))
                    if lk:
                        late.append(lk.pop(0))
            else:
                for st in range(TPB, ST):
                    sched(1, (2 * st) % TPB, v_unit(st))

            # ------------- output projection for one s-tile ---------------
            def emit_outproj_st(st, act_share, split_dma=False):
                o_sb = ostage.tile([128, D], bf16, tag="o")
                for n in range(D // 512):
                    ops = scrpool.tile([128, 512], f32, tag="scr")
                    for p2 in range(MQ):
                        nc.tensor.matmul(
                            ops[:], ctx_sb[:, p2, st * 128:(st + 1) * 128],
                            wo_sb[:, p2, n * 512:(n + 1) * 512],
                            start=(p2 == 0), stop=(p2 == MQ - 1),
                        )
                    sl = slice(n * 512, (n + 1) * 512)
                    if n < D // 512 - act_share:
                        nc.vector.tensor_copy(o_sb[:, sl], ops[:])
                    else:
                        nc.scalar.activation(o_sb[:, sl], ops[:], A.Copy)
                    if split_dma:
                        nc.sync.dma_start(out[st * 128:(st + 1) * 128, sl],
                                          o_sb[:, sl])
                if not split_dma:
                    nc.sync.dma_start(out[st * 128:(st + 1) * 128, :], o_sb[:])

            # ------------- attention for one (j, h) -----------------------
            outproj_q = deque()

            def emit_burst(j, h, qc, bank):
                """Flipped-PV burst for query-tile qc + softmax normalize;
                on the second head of a pair also transpose ctx to ctxT and
                queue the s-tile's output projection."""
                hm, po = h // 2, 64 * (h % 2)
                qt = j * TPB + qc
                ctx_ps = cpool.tile([128, DH + 1], f32, tag="ctx")
                for s2 in range(qt + 1):
                    nc.tensor.matmul(
                        ctx_ps[:],
                        attn_sb[:, s2, bank, qc * 128:(qc + 1) * 128],
                        v_sb[:, s2, h, :],
                        start=(s2 == 0), stop=(s2 == qt),
                    )
                dn = rpool.tile([128, 1], f32, tag="dn")
                rc = rpool.tile([128, 1], f32, tag="rc")
                nc.vector.tensor_copy(dn[:], ctx_ps[:, DH:DH + 1])
                # custom-DVE reciprocal must read SBUF (psum source returns
                # garbage on hardware); ~51 ULP is negligible.  A direct
                # tensor_scalar divide fails the neuron codegen ISA check.
                nc.vector.reciprocal_approx_fast(out=rc[:], in_=dn[:])
                nc.vector.tensor_scalar(
                    ctxn_sb[:, hm, qc, po:po + DH], ctx_ps[:, 0:DH],
                    rc[:], None, Alu.mult,
                )
                if po:  # both heads of the pair done for this qc
                    tp = scrpool.tile([128, 512], f32, tag="scr")
                    tpb = tp[:, 0:64].bitcast(bf16)
                    nc.tensor.transpose(tpb, ctxn_sb[:, hm, qc, :], ident[:])
                    nc.vector.tensor_copy(
                        ctx_sb[:, hm, qt * 128:(qt + 1) * 128], tpb,
                    )
                    if hm == MQ - 1:
                        outproj_q.append(qt)

            def attention_pass(j, hs, pass_idx, look=False):
                """One ski sweep emitting two heads per step (one per
                attn bank/head-pair); bursts lag their diagonal exp by one
                step so the PE never queues a burst ahead of the next
                scores.  attn banks rotate mod 3 so a pass's first exps
                never WAR-wait on the previous pass's last bursts."""
                banks = {hh: (2 * pass_idx + i) % 3 for i, hh in enumerate(hs)}
                nski = TPB * (j + 1)

                def se(ski):
                    ex0 = max(0, 128 * ski - j * W)
                    for h in hs:
                        hm, po = h // 2, 64 * (h % 2)
                        qrow = slice(po, po + DH)
                        sc = spool.tile([128, W], f32, tag="sc")
                        for n in range(ex0 // 512, W // 512):
                            lo = max(ex0, n * 512)
                            nc.tensor.matmul(
                                sc[:, lo:(n + 1) * 512],
                                kT_sb[qrow, hm, ski * 128:(ski + 1) * 128],
                                qT_sb[qrow, hm,
                                      j * W + lo: j * W + (n + 1) * 512],
                                start=True, stop=True,
                            )
                        at = attn_sb[:, banks[h], ski]
                        nc.scalar.activation(at[:, ex0:W], sc[:, ex0:W],
                                             A.Exp, scale=scale)
                        if 128 * ski >= j * W:  # diagonal-crossing tile
                            nc.gpsimd.affine_select(
                                out=at[:, ex0:ex0 + 128],
                                in_=at[:, ex0:ex0 + 128],
                                compare_op=Alu.is_ge, fill=0.0,
                                base=j * W + ex0 - 128 * ski,
                                pattern=[[1, 128]], channel_multiplier=-1,
                            )

                for ski in range(nski):
                    # "pre" fillers (projection units) must precede the step
                    # (write-before-read in the PE queue); "post" fillers
                    # (output projections) go behind the step's scores so
                    # they never delay the exp stream.  The final pass runs
                    # its scores one step ahead so the shrinking last exps
                    # never trail the growing burst/projection chains.
                    yield "pre", ski
                    if not look:
                        se(ski)
                    else:
                        if ski == 0:
                            se(0)
                        if ski + 1 < nski:
                            se(ski + 1)
                    if ski > j * TPB:
                        for h in hs:
                            emit_burst(j, h, ski - 1 - j * TPB, banks[h])
                    if ski == nski - 1 and not last_pass:
                        # the first head's bank is reused by the next pass's
                        # exps: retire its last burst inside the loop
                        emit_burst(j, hs[0], TPB - 1, banks[hs[0]])
                    yield "post", ski
                if last_pass:
                    emit_burst(j, hs[0], TPB - 1, banks[hs[0]])
                emit_burst(j, hs[1], TPB - 1, banks[hs[1]])

            for j in range(NJ):
                if j > 0:
                    for fns in [j0sched.pop(k) for k in sorted(j0sched)]:
                        for fn in fns:
                            fn()
                nski_j = TPB * (j + 1)
                for p in range(2):
                    hs = (0, 2) if p == 0 else (1, 3)
                    if j > 0 and p == 1:
                        while late:
                            late.popleft()()
                    popped = False
                    for phase, ski in attention_pass(
                        j, hs, 2 * j + p, look=(j == NJ - 1 and p == 1),
                    ):
                        if phase == "pre":
                            popped = False
                        elif j == 0 and (p, ski) in j0sched:
                            popped = True
                            for fn in j0sched.pop((p, ski)):
                                fn()
                        elif j > 0 and p == 0 and late:
                            popped = True
                            late.popleft()()
                        elif not popped and outproj_q and (
                            (j == NJ - 1 and p == 1 and ski < nski_j - 3)
                            or (j > 0 and ski % 3 == 2)
                        ):
                            tail = j == NJ - 1 and p == 1 and ski >= nski_j - 6
                            emit_outproj_st(outproj_q.popleft(),
                                            act_share=1 if tail else 0)
            while late:
                late.popleft()()
            while outproj_q:
                emit_outproj_st(outproj_q.popleft(), act_share=1,
                                split_dma=len(outproj_q) == 0)
            ctx_cm.__exit__(None, None, None)
            sc_cm.__exit__(None, None, None)
            scr_cm.__exit__(None, None, None)

    nc.compile()
    return nc


def _get_program():
    key = (S, D, HL, DH)
    if key not in _PROGRAM_CACHE:
        _PROGRAM_CACHE[key] = build_program(*key)
    return _PROGRAM_CACHE[key]


def prepare_core_inputs(xT, wq, wk, wv, wo):
    """fp32 per-core inputs (xT [D,S], wq/wk/wv [D,M], wo [M,D]) -> the
    device tensor map (bf16 + prescaled-fp8 q/k path)."""
    import ml_dtypes

    bf16 = ml_dtypes.bfloat16
    f8 = ml_dtypes.float8_e4m3
    xT = np.ascontiguousarray(xT, dtype=np.float32)
    wqk = np.concatenate([np.asarray(wq), np.asarray(wk)], axis=1) * W_SCALE
    return {
        "xT": xT.astype(bf16),
        "x8T": xT.astype(f8),
        "wqk": np.ascontiguousarray(wqk).astype(f8),
        "wv": np.ascontiguousarray(wv).astype(bf16),
        "wo": np.ascontiguousarray(wo).astype(bf16),
    }


def make_in_maps(x, Wq, Wk, Wv, Wo):
    x = np.asarray(x, dtype=np.float32)
    Wq = np.asarray(Wq, dtype=np.float32)
    Wk = np.asarray(Wk, dtype=np.float32)
    Wv = np.asarray(Wv, dtype=np.float32)
    Wo = np.asarray(Wo, dtype=np.float32)
    in_maps = []
    for c in range(NCORES):
        b, g = divmod(c, NCORES // B)
        sl = slice(HL * DH * g, HL * DH * (g + 1))
        in_maps.append(
            prepare_core_inputs(x[b].T, Wq[sl, :].T, Wk[sl, :].T,
                                Wv[sl, :].T, Wo[:, sl].T)
        )
    return in_maps


def kernel(x, Wq, Wk, Wv, Wo, bo):
    from concourse import bass2jax

    nc = _get_program()
    in_maps = make_in_maps(x, Wq, Wk, Wv, Wo)
    res = bass2jax.run_bass_via_pjrt(nc, in_maps, n_cores=NCORES)
    outs = [np.asarray(res[c]["out"], dtype=np.float32) for c in range(NCORES)]
    gpb = NCORES // B
    o = np.stack([sum(outs[b * gpb + g] for g in range(gpb)) for b in range(B)])
    o = o + np.asarray(bo, dtype=np.float32)[None, None, :]
    return o.astype(np.float32)


# revision 96
# speedup vs baseline: 1.0574x; 1.0005x over previous
"""Trainium2 Bass kernel: multi-head attention forward (B=2, S=2048, D=1024, H=16).

Sharding: 8 cores = data-parallel over batch (2) x tensor-parallel over heads
(4 head-groups of 4 heads).  Host sums the 4 partial outputs per batch and
adds the bias.

Per-core algorithm (all matmul operands bf16: 1 cycle/row at any width):
  qT/kT = w.T @ x.T        [256, S]   (per-head-group projections)
  v     = x @ wv           [S, 256]   (+ a ones column per head for the
                                       softmax denominators)
  per (query-block j, head h, key-tile ski):
      scoresT[sk, sq] = k_h @ q_h.T   (psum, 512-wide chunks, causal-trimmed)
      attnT = exp(scale * scoresT)    (ACT, psum -> SBUF bf16)
      causal mask on the diagonal tile (GPSIMD affine_select, fill 0)
  per query-tile qc (once its diagonal exp is done):
      ctx'[sq, 65] = sum_ski attnT_tile.T @ [v_h | 1]   (keys contracted on
          partitions, queries on output partitions: 65 cycles per key-tile
          instead of the 512 a [65, sq]-oriented PV pays)
      recip = 1/ctx'[:, 64]           (DVE approx; SBUF source only)
      ctx_n[sq, 64] = ctx' * recip    (per-partition scalar, bf16)
  per head-pair: PE-transpose ctx_n [sq,128] -> ctxT [128, sq] for the
      output projection (row-sharded wo), emitted per 128-row s-tile so the
      projection and its DMA overlap the remaining attention.
"""

import sys

sys.path.insert(0, "/opt/trn_rl_repo")

import numpy as np

B, S, D = 2, 2048, 1024
H = 16
DH = 64
HL = 4  # heads per core
NCORES = 8
W_SCALE = 32.0  # fp8 weight prescale so w ~ N(0, 0.02) clears e4m3 denormals

_PROGRAM_CACHE = {}


def build_program(S=S, D=D, HL=HL, DH=DH):
    from collections import deque

    import concourse.tile as tile
    from concourse import bacc, mybir

    f32 = mybir.dt.float32
    bf16 = mybir.dt.bfloat16
    f8 = mybir.dt.float8e4
    A = mybir.ActivationFunctionType
    Alu = mybir.AluOpType
    DR = mybir.MatmulPerfMode.DoubleRow

    KD = D // 128        # contraction chunks for the projections
    KD4 = D // 256       # fp8 DoubleRow contraction chunks (2 rows/partition)
    M = HL * DH          # per-core projected width (256)
    MQ = M // 128        # qT/kT partition tiles (2)
    ST = S // 128        # 128-row s tiles
    W = min(1024, S)     # query-block width
    NJ = S // W          # query blocks
    TPB = W // 128       # 128-tiles per query block
    NCH = S // 512       # 512-wide s chunks
    GA_N = min(2, NCH)   # s chunks covered by the k-outer warmup group
    N_WARMUP = 12 if NCH >= 4 else 0
    scale = 1.0 / float(np.sqrt(DH))

    nc = bacc.Bacc("TRN2", target_bir_lowering=False, debug=False)
    xT = nc.dram_tensor("xT", (D, S), bf16, kind="ExternalInput").ap()
    # q/k path in fp8e4m3: weights host-prescaled by W_SCALE, undone in the
    # psum->sbuf copies; v/out stay bf16 (fp8 there fails the 2e-2 gate).
    # wq|wk are host-interleaved into one tensor so the DMA rows reach the
    # 512B no-penalty descriptor size.
    x8T = nc.dram_tensor("x8T", (D, S), f8, kind="ExternalInput").ap()
    wqk = nc.dram_tensor("wqk", (D, 2 * M), f8, kind="ExternalInput").ap()
    wv = nc.dram_tensor("wv", (D, M), bf16, kind="ExternalInput").ap()
    wo = nc.dram_tensor("wo", (M, D), bf16, kind="ExternalInput").ap()
    out = nc.dram_tensor("out", (S, D), bf16, kind="ExternalOutput").ap()

    with tile.TileContext(nc) as tc:
        with (
            tc.tile_pool(name="persist", bufs=1) as mpool,
            tc.tile_pool(name="ostage", bufs=6) as ostage,
            tc.tile_pool(name="rp", bufs=2) as rpool,
        ):
            wo_sb = mpool.tile([128, MQ, D], bf16, tag="wo")
            qT_sb = mpool.tile([128, MQ, S], bf16, tag="qT")
            kT_sb = mpool.tile([128, MQ, S], bf16, tag="kT")
            v_sb = mpool.tile([128, ST, HL, DH + 1], bf16, tag="v")
            ctx_sb = mpool.tile([128, MQ, S], bf16, tag="ctx")
            # bank dim innermost-but-one so a paired two-bank activation
            # AP's bank stride (W elems) fits the 16-bit ISA stride field
            attn_sb = mpool.tile([128, ST, 3, W], bf16, tag="attn")
            ctxn_sb = mpool.tile([128, MQ, TPB, 128], bf16, tag="ctxn")
            ident = mpool.tile([128, 128], bf16, tag="id")
            xt = mpool.tile([128, KD, S], bf16, tag="xt")
            x8 = mpool.tile([128, KD4, 2, S], f8, tag="x8")
            wqk_sb = mpool.tile([128, KD4, 2, 2 * M], f8, tag="wqk")
            wv_sb = mpool.tile([128, KD, M], bf16, tag="wv")

            nc.gpsimd.memset(ident[:], 0.0)
            nc.gpsimd.affine_select(
                out=ident[:], in_=ident[:], compare_op=Alu.not_equal,
                fill=1.0, base=0, pattern=[[-1, 128]], channel_multiplier=1,
            )
            # ones columns for the PV denominator trick
            nc.gpsimd.memset(v_sb[:, :, :, DH], 1.0)

            # ---------------- loads (order gates the exp-stream start) ----
            wqk_r = wqk.rearrange("(c i p) m -> p c i m", p=128, i=2)
            wv_r = wv.rearrange("(k p) m -> p k m", p=128)
            xT_r = xT.rearrange("(k p) s -> p k s", p=128)
            x8_r = x8T.rearrange("(c i p) s -> p c i s", p=128, i=2)
            nhalf = 2 if NCH >= 4 else 1
            xh = S // nhalf
            nq = 4 if NCH >= 4 else 1
            xq = S // nq
            nc.sync.dma_start(wqk_sb[:], wqk_r[:])
            nc.sync.dma_start(x8[:, 0:KD4 // 2, :, 0:xh],
                              x8_r[:, 0:KD4 // 2, :, 0:xh])
            nc.sync.dma_start(x8[:, KD4 // 2:KD4, :, 0:xh],
                              x8_r[:, KD4 // 2:KD4, :, 0:xh])
            nc.sync.dma_start(xt[:, :, 0:128], xT_r[:, :, 0:128])
            nc.sync.dma_start(wv_sb[:], wv_r[:])
            nc.sync.dma_start(xt[:, :, 128:xq], xT_r[:, :, 128:xq])
            if nq > 1:
                nc.sync.dma_start(xt[:, :, xq:2 * xq], xT_r[:, :, xq:2 * xq])
            if nhalf > 1:
                nc.sync.dma_start(x8[:, :, :, xh:S], x8_r[:, :, :, xh:S])
            if nq > 1:
                nc.sync.dma_start(xt[:, :, 2 * xq:3 * xq],
                                  xT_r[:, :, 2 * xq:3 * xq])
                nc.sync.dma_start(xt[:, :, 3 * xq:S], xT_r[:, :, 3 * xq:S])
            nc.sync.dma_start(wo_sb[:], wo.rearrange("(k p) d -> p k d", p=128))

            # ---------- warmup projections (c-outer over GA_N chunks) -----
            # q/k for all heads, s < GA_N*512: enough to start attention
            # (j=0, both passes) as soon as the fp8 first half of x lands.
            n_ga = (GA_N + 1) * MQ
            wu_cm = tc.tile_pool(name="wups", bufs=1, space="PSUM")
            wupool = wu_cm.__enter__()
            wu = wupool.tile([128, 128], bf16, tag="wu")
            # dummy transposes keep the PE continuously busy through the
            # input DMA so the warmup projections run at full p-state
            for _ in range(N_WARMUP):
                nc.tensor.transpose(wu[:], ident[:], ident[:])
            wu_cm.__exit__(None, None, None)
            ga_cm = tc.tile_pool(name="gaps", bufs=n_ga, space="PSUM")
            gapool = ga_cm.__enter__()
            ga = [gapool.tile([128, 512], f32, tag="ga", name=f"ga{i}")
                  for i in range(n_ga)]
            # first-half-of-x chunks sweep across all tiles (DMA-paced),
            # then each tile finishes its second half and copies out
            # immediately, so the copies gating the first exps retire early
            descale = 1.0 / W_SCALE
            ga_tiles = []
            i = 0
            for m in range(MQ):
                for n in range(GA_N):
                    sl = slice(n * 512, (n + 1) * 512)
                    ga_tiles.append((ga[i], m * 128, qT_sb, m, sl, True))
                    i += 1
                    if n == 0:  # later kT chunks are deferred units
                        ga_tiles.append((ga[i], M + m * 128, kT_sb, m, sl,
                                         False))
                        i += 1
            for c in range(KD4 // 2):
                for t, col, dst, m, sl, is_q in ga_tiles:
                    nc.tensor.matmul(t[:], wqk_sb[:, c, :, col:col + 128],
                                     x8[:, c, :, sl], perf_mode=DR,
                                     start=(c == 0), stop=False)
            for t, col, dst, m, sl, is_q in ga_tiles:
                for c in range(KD4 // 2, KD4):
                    nc.tensor.matmul(t[:], wqk_sb[:, c, :, col:col + 128],
                                     x8[:, c, :, sl], perf_mode=DR,
                                     start=False, stop=(c == KD4 - 1))
                if is_q:
                    nc.vector.tensor_scalar(dst[:, m, sl], t[:],
                                            descale, None, Alu.mult)
                else:
                    nc.scalar.activation(dst[:, m, sl], t[:], A.Copy,
                                         scale=descale)
            ga_cm.__exit__(None, None, None)

            # --------------- attention-phase psum pools -------------------
            # entry order fixes bank placement: the scratch ring lands on the
            # banks whose warmup copies retire first, the scores ring next
            scr_cm = tc.tile_pool(name="scrps", bufs=2, space="PSUM")
            scrpool = scr_cm.__enter__()
            sc_cm = tc.tile_pool(name="scps", bufs=2, space="PSUM")
            spool = sc_cm.__enter__()
            ctx_cm = tc.tile_pool(name="ctxps", bufs=2, space="PSUM")
            cpool = ctx_cm.__enter__()

            # ------------- deferred projection units (one psum tile) ------
            def qk_unit(base, dst, m, n):
                def emit():
                    ps = scrpool.tile([128, 512], f32, tag="scr")
                    sl = slice(n * 512, (n + 1) * 512)
                    col = base + m * 128
                    for c in range(KD4):
                        nc.tensor.matmul(ps[:], wqk_sb[:, c, :, col:col + 128],
                                         x8[:, c, :, sl], perf_mode=DR,
                                         start=(c == 0), stop=(c == KD4 - 1))
                    nc.vector.tensor_scalar(dst[:, m, sl], ps[:],
                                            1.0 / W_SCALE, None, Alu.mult)
                return emit

            def v_unit(st):
                def emit():
                    ps = scrpool.tile([128, 512], f32, tag="scr")
                    for k in range(KD):
                        nc.tensor.matmul(ps[:, 0:M], xt[:, k, st * 128:(st + 1) * 128],
                                         wv_sb[:, k, :], start=(k == 0),
                                         stop=(k == KD - 1))
                    nc.vector.tensor_copy(
                        v_sb[:, st, :, 0:DH],
                        ps[:, 0:M].rearrange("p (h c) -> p h c", h=HL),
                    )
                return emit

            # early units feed j=0 (and j=1's first scores); late units are
            # only needed from j=1's deeper ski range on and fill j=1's
            # PE slack while ACT paces the exp stream
            # j=0 unit schedule keyed by (pass, ski), honoring deadlines:
            # kT n>=1 chunks before scores ski=4n, v(qc) before its burst at
            # step qc+1, qT chunks for j=1 anytime within pass B
            j0sched = {}

            def sched(p, ski, fn):
                j0sched.setdefault((p, ski), []).append(fn)

            for n in range(1, GA_N):
                for m in range(MQ):
                    sched(0, 0, qk_unit(M, kT_sb, m, n))
            # post-slot pops: v(qc) lands after step qc's emission, ahead
            # of its burst in step qc+1 (the last tile's burst can retire
            # in-loop, so its v leads by one)
            for qc in range(min(ST, TPB)):
                sched(0, min(qc, TPB - 2), v_unit(qc))
            qn = [qk_unit(0, qT_sb, m, n)
                  for n in range(GA_N, NCH) for m in range(MQ)]
            for i, fn in enumerate(qn):
                sched(1, (2 * i) % TPB, fn)
            late = deque()
            if NJ > 1:
                # interleave the late v tiles with the late-needed kT
                # chunks: v(TPB+i) is read from burst step TPB+i on, kT
                # chunk n from scores step 4n on
                lv = [v_unit(st) for st in range(TPB, ST)]
                lk = [qk_unit(M, kT_sb, m, n)
                      for n in range(GA_N, NCH) for m in range(MQ)]
                while lv or lk:
                    if lv:
                        late.append(lv.pop(0))
                    if lk:
                        late.append(lk.pop(0))
            else:
                for st in range(TPB, ST):
                    sched(1, (2 * st) % TPB, v_unit(st))

            # ------------- output projection for one s-tile ---------------
            def emit_outproj_st(st, act_share, split_dma=False):
                o_sb = ostage.tile([128, D], bf16, tag="o")
                for n in range(D // 512):
                    ops = scrpool.tile([128, 512], f32, tag="scr")
                    for p2 in range(MQ):
                        nc.tensor.matmul(
                            ops[:], ctx_sb[:, p2, st * 128:(st + 1) * 128],
                            wo_sb[:, p2, n * 512:(n + 1) * 512],
                            start=(p2 == 0), stop=(p2 == MQ - 1),
                        )
                    sl = slice(n * 512, (n + 1) * 512)
                    if n < D // 512 - act_share:
                        nc.vector.tensor_copy(o_sb[:, sl], ops[:])
                    else:
                        nc.scalar.activation(o_sb[:, sl], ops[:], A.Copy)
                    if split_dma:
                        nc.sync.dma_start(out[st * 128:(st + 1) * 128, sl],
                                          o_sb[:, sl])
                if not split_dma:
                    nc.sync.dma_start(out[st * 128:(st + 1) * 128, :], o_sb[:])

            # ------------- attention for one (j, h) -----------------------
            outproj_q = deque()

            def emit_burst(j, h, qc, bank):
                """Flipped-PV burst for query-tile qc + softmax normalize;
                on the second head of a pair also transpose ctx to ctxT and
                queue the s-tile's output projection."""
                hm, po = h // 2, 64 * (h % 2)
                qt = j * TPB + qc
                ctx_ps = cpool.tile([128, DH + 1], f32, tag="ctx")
                for s2 in range(qt + 1):
                    nc.tensor.matmul(
                        ctx_ps[:],
                        attn_sb[:, s2, bank, qc * 128:(qc + 1) * 128],
                        v_sb[:, s2, h, :],
                        start=(s2 == 0), stop=(s2 == qt),
                    )
                dn = rpool.tile([128, 1], f32, tag="dn")
                rc = rpool.tile([128, 1], f32, tag="rc")
                nc.vector.tensor_copy(dn[:], ctx_ps[:, DH:DH + 1])
                # custom-DVE reciprocal must read SBUF (psum source returns
                # garbage on hardware); ~51 ULP is negligible.  A direct
                # tensor_scalar divide fails the neuron codegen ISA check.
                nc.vector.reciprocal_approx_fast(out=rc[:], in_=dn[:])
                nc.vector.tensor_scalar(
                    ctxn_sb[:, hm, qc, po:po + DH], ctx_ps[:, 0:DH],
                    rc[:], None, Alu.mult,
                )
                if po:  # both heads of the pair done for this qc
                    tp = scrpool.tile([128, 512], f32, tag="scr")
                    tpb = tp[:, 0:64].bitcast(bf16)
                    nc.tensor.transpose(tpb, ctxn_sb[:, hm, qc, :], ident[:])
                    nc.vector.tensor_copy(
                        ctx_sb[:, hm, qt * 128:(qt + 1) * 128], tpb,
                    )
                    if hm == MQ - 1:
                        outproj_q.append(qt)

            def attention_pass(j, hs, pass_idx, look=False):
                """One ski sweep emitting two heads per step (one per
                attn bank/head-pair); bursts lag their diagonal exp by one
                step so the PE never queues a burst ahead of the next
                scores.  attn banks rotate mod 3 so a pass's first exps
                never WAR-wait on the previous pass's last bursts."""
                banks = {hh: (2 * pass_idx + i) % 3 for i, hh in enumerate(hs)}
                nski = TPB * (j + 1)

                def se(ski):
                    ex0 = max(0, 128 * ski - j * W)
                    for h in hs:
                        hm, po = h // 2, 64 * (h % 2)
                        qrow = slice(po, po + DH)
                        sc = spool.tile([128, W], f32, tag="sc")
                        for n in range(ex0 // 512, W // 512):
                            lo = max(ex0, n * 512)
                            nc.tensor.matmul(
                                sc[:, lo:(n + 1) * 512],
                                kT_sb[qrow, hm, ski * 128:(ski + 1) * 128],
                                qT_sb[qrow, hm,
                                      j * W + lo: j * W + (n + 1) * 512],
                                start=True, stop=True,
                            )
                        at = attn_sb[:, banks[h], ski]
                        nc.scalar.activation(at[:, ex0:W], sc[:, ex0:W],
                                             A.Exp, scale=scale)
                        if 128 * ski >= j * W:  # diagonal-crossing tile
                            nc.gpsimd.affine_select(
                                out=at[:, ex0:ex0 + 128],
                                in_=at[:, ex0:ex0 + 128],
                                compare_op=Alu.is_ge, fill=0.0,
                                base=j * W + ex0 - 128 * ski,
                                pattern=[[1, 128]], channel_multiplier=-1,
                            )

                for ski in range(nski):
                    # "pre" fillers (projection units) must precede the step
                    # (write-before-read in the PE queue); "post" fillers
                    # (output projections) go behind the step's scores so
                    # they never delay the exp stream.  The final pass runs
                    # its scores one step ahead so the shrinking last exps
                    # never trail the growing burst/projection chains.
                    yield "pre", ski
                    if not look:
                        se(ski)
                    else:
                        if ski == 0:
                            se(0)
                        if ski + 1 < nski:
                            se(ski + 1)
                    if ski > j * TPB:
                        for h in hs:
                            emit_burst(j, h, ski - 1 - j * TPB, banks[h])
                    if ski == nski - 1 and not last_pass:
                        # the first head's bank is reused by the next pass's
                        # exps: retire its last burst inside the loop
                        emit_burst(j, hs[0], TPB - 1, banks[hs[0]])
                    yield "post", ski
                if last_pass:
                    emit_burst(j, hs[0], TPB - 1, banks[hs[0]])
                emit_burst(j, hs[1], TPB - 1, banks[hs[1]])

            for j in range(NJ):
                if j > 0:
                    for fns in [j0sched.pop(k) for k in sorted(j0sched)]:
                        for fn in fns:
                            fn()
                nski_j = TPB * (j + 1)
                for p in range(2):
                    hs = (0, 2) if p == 0 else (1, 3)
                    if j > 0 and p == 1:
                        while late:
                            late.popleft()()
                    popped = False
                    for phase, ski in attention_pass(
                        j, hs, 2 * j + p, look=(j == NJ - 1 and p == 1),
                    ):
                        if phase == "pre":
                            popped = False
                        elif j == 0 and (p, ski) in j0sched:
                            popped = True
                            for fn in j0sched.pop((p, ski)):
                                fn()
                        elif j > 0 and p == 0 and late:
                            popped = True
                            late.popleft()()
                        elif not popped and outproj_q and (
                            (j == NJ - 1 and p == 1 and ski < nski_j - 3)
                            or (j > 0 and ski % 3 == 2)
                        ):
                            tail = j == NJ - 1 and p == 1 and ski >= nski_j - 6
                            emit_outproj_st(outproj_q.popleft(),
                                            act_share=1 if tail else 0)
            while late:
                late.popleft()()
            while outproj_q:
                emit_outproj_st(outproj_q.popleft(), act_share=1,
                                split_dma=len(outproj_q) == 0)
            ctx_cm.__exit__(None, None, None)
            sc_cm.__exit__(None, None, None)
            scr_cm.__exit__(None, None, None)

    nc.compile()
    return nc


def _get_program():
    key = (S, D, HL, DH)
    if key not in _PROGRAM_CACHE:
        _PROGRAM_CACHE[key] = build_program(*key)
    return _PROGRAM_CACHE[key]


def prepare_core_inputs(xT, wq, wk, wv, wo):
    """fp32 per-core inputs (xT [D,S], wq/wk/wv [D,M], wo [M,D]) -> the
    device tensor map (bf16 + prescaled-fp8 q/k path)."""
    import ml_dtypes

    bf16 = ml_dtypes.bfloat16
    f8 = ml_dtypes.float8_e4m3
    xT = np.ascontiguousarray(xT, dtype=np.float32)
    wqk = np.concatenate([np.asarray(wq), np.asarray(wk)], axis=1) * W_SCALE
    return {
        "xT": xT.astype(bf16),
        "x8T": xT.astype(f8),
        "wqk": np.ascontiguousarray(wqk).astype(f8),
        "wv": np.ascontiguousarray(wv).astype(bf16),
        "wo": np.ascontiguousarray(wo).astype(bf16),
    }


def make_in_maps(x, Wq, Wk, Wv, Wo):
    x = np.asarray(x, dtype=np.float32)
    Wq = np.asarray(Wq, dtype=np.float32)
    Wk = np.asarray(Wk, dtype=np.float32)
    Wv = np.asarray(Wv, dtype=np.float32)
    Wo = np.asarray(Wo, dtype=np.float32)
    in_maps = []
    for c in range(NCORES):
        b, g = divmod(c, NCORES // B)
        sl = slice(HL * DH * g, HL * DH * (g + 1))
        in_maps.append(
            prepare_core_inputs(x[b].T, Wq[sl, :].T, Wk[sl, :].T,
                                Wv[sl, :].T, Wo[:, sl].T)
        )
    return in_maps


def kernel(x, Wq, Wk, Wv, Wo, bo):
    from concourse import bass2jax

    nc = _get_program()
    in_maps = make_in_maps(x, Wq, Wk, Wv, Wo)
    res = bass2jax.run_bass_via_pjrt(nc, in_maps, n_cores=NCORES)
    outs = [np.asarray(res[c]["out"], dtype=np.float32) for c in range(NCORES)]
    gpb = NCORES // B
    o = np.stack([sum(outs[b * gpb + g] for g in range(gpb)) for b in range(B)])
    o = o + np.asarray(bo, dtype=np.float32)[None, None, :]
    return o.astype(np.float32)
